# revision 9
# baseline (speedup 1.0000x reference)
"""Biaffine NER model (2-layer BiLSTM + highway + biaffine) on 8 Trainium2 cores.

Strategy:
  - Data-parallel over batch: each of the 8 cores handles B_loc=2 of the 16
    batch elements, full model, no collectives.
  - The LSTM recurrences are solved by fixed-point (Jacobi) iteration:
      H^{k+1} = LSTMCell(x_tilde + shift(H^k) @ W_h)
    Each iteration is fully parallel over time (big matmuls, M = B_loc*T = 512
    rows), and the cell-state recurrence c_t = a_t*c_{t-1} + b_t is computed
    with the hardware tensor_tensor_scan. The map contracts by ~4x per
    iteration; K_ITERS=5 sits at ~9.5e-3 rel absmax vs the 2e-2 gate.
  - Everything on-chip is kept "transposed" (feature-major, [128-partition
    folds, (b, t) free]) so matmuls, activations and scans all operate on
    full-width tiles.
  - All contraction dims are zero-padded to multiples of 128 host-side so
    every matmul uses uniform full-128 K-tiles (padding rows are zero on both
    the stationary and moving side).
  - Elementwise gate math runs in bf16 (DVE 2x/4x perf modes).
  - Biases ride as an extra contraction row (ones rail in the moving operand,
    bias row in the stationary operand).
  - Output is DMA'd as bf16 and upcast host-side (halves the output-write
    tail; adds <4e-4 rel err).
"""

import sys

sys.path.insert(0, "/opt/trn_rl_repo")

import ml_dtypes
import numpy as np

import concourse.bass as bass
import concourse.mybir as mybir
import concourse.tile as tile
from concourse.bass_utils import run_bass_kernel_spmd
from concourse.masks import make_identity

F32 = mybir.dt.float32
BF16 = mybir.dt.bfloat16
BF16NP = ml_dtypes.bfloat16
AF = mybir.ActivationFunctionType
ALU = mybir.AluOpType

B, T, D = 16, 256, 768
H, H2, G = 400, 800, 1200
F, C = 150, 8
NCORES = 8
BL = B // NCORES          # 2 batch elements per core
L = BL * T                # 512 (b, t) rows per core
GP = 512                  # per-gate padded stride (3*GP = 1536, 12 M-tiles)
NM = 12                   # M-tiles of the padded gate dim
NKH = 4                   # K-tiles of the padded [H+1->512] contraction
NKD = 6                   # K-tiles of D=768
K_ITERS = 5

_CACHE = {}


# ------------------------------------------------------------------ host packing

def _pack_gate_cols(w):
    """[K, 3H] -> [K, 3*GP] with each gate's 400 cols padded to 512."""
    k = w.shape[0]
    out = np.zeros((k, 3 * GP), np.float32)
    for g in range(3):
        out[:, g * GP:g * GP + H] = w[:, g * H:(g + 1) * H]
    return out


def _with_bias_row(w, bias):
    """Append one row (the bias, packed like w's columns) to w."""
    return np.concatenate([w, bias[None, :]], 0)


def _fold_k(w, nk):
    """[K<=128*nk, C] -> [128, nk, C] zero-padded row fold (row r -> [r%128, r//128])."""
    k, c = w.shape
    out = np.zeros((128 * nk, c), np.float32)
    out[:k] = w
    return np.ascontiguousarray(out.reshape(nk, 128, c).transpose(1, 0, 2))


def _fold128(v, nchunk):
    """[n] -> [128, nchunk] column-major fold (unit u -> [u%128, u//128])."""
    out = np.zeros((128, nchunk), np.float32)
    n = len(v)
    for m in range(nchunk):
        seg = v[m * 128:min((m + 1) * 128, n)]
        out[:len(seg), m] = seg
    return out


def _pack_inputs(inputs):
    """Pack weights into the DRAM layouts the program expects (shared by all cores)."""
    f32 = lambda a: np.ascontiguousarray(np.asarray(a, np.float32))
    x = f32(inputs["x"])
    z = np.zeros((3 * GP,), np.float32)

    packs = {}
    # layer 0: W [D+H, 3H].  The gate bias rides as the last row of wh (it is
    # re-added every Jacobi iteration through the ones slot of ht).
    for nm, wn, bn in (("0f", "W_f0", "b_f0"), ("0b", "W_b0", "b_b0")):
        W = f32(inputs[wn]); bias = _pack_gate_cols(f32(inputs[bn])[None, :])[0]
        packs["wx" + nm] = _fold_k(_pack_gate_cols(W[:D]), NKD)
        packs["wh" + nm] = _fold_k(_with_bias_row(_pack_gate_cols(W[D:]), bias), NKH)
    # layer 1: W [2H+H, 3H]; the input half splits into hf/hb parts (both with
    # zero bias rows -- the bias lives only in wh).
    for nm, wn, bn in (("1f", "W_f1", "b_f1"), ("1b", "W_b1", "b_b1")):
        W = f32(inputs[wn]); bias = _pack_gate_cols(f32(inputs[bn])[None, :])[0]
        packs["wx" + nm + "f"] = _fold_k(_with_bias_row(_pack_gate_cols(W[:H]), z), NKH)
        packs["wx" + nm + "b"] = _fold_k(_with_bias_row(_pack_gate_cols(W[H:H2]), z), NKH)
        packs["wh" + nm] = _fold_k(_with_bias_row(_pack_gate_cols(W[H2:]), bias), NKH)

    # highway: W_hw [2H, 2H]; M packed as [f-half pad 512 | b-half pad 512]
    Whw = f32(inputs["W_hw"]); bhw = f32(inputs["b_hw"])

    def _pack_hw_cols(w):
        k = w.shape[0]
        out = np.zeros((k, 2 * GP), np.float32)
        out[:, 0:H] = w[:, 0:H]
        out[:, GP:GP + H] = w[:, H:H2]
        return out

    zh = np.zeros((2 * GP,), np.float32)
    packs["whwf"] = _fold_k(_with_bias_row(_pack_hw_cols(Whw[:H]), _pack_hw_cols(bhw[None, :])[0]), NKH)
    packs["whwb"] = _fold_k(_with_bias_row(_pack_hw_cols(Whw[H:]), zh), NKH)

    # projections: Ws/We [2H, F]
    for nm, wn, bn in (("s", "W_s", "b_s"), ("e", "W_e", "b_e")):
        W = f32(inputs[wn]); bias = f32(inputs[bn])
        packs["w" + nm + "f"] = _fold_k(_with_bias_row(W[:H], bias), NKH)
        packs["w" + nm + "b"] = _fold_k(_with_bias_row(W[H:], np.zeros((F,), np.float32)), NKH)

    # biaffine U [F+1, C, F+1] -> [F+1, C*256] (each c padded 151->256)
    U = f32(inputs["U"])
    upk = np.zeros((F + 1, C * 256), np.float32)
    for c in range(C):
        upk[:, c * 256:c * 256 + F + 1] = U[:, c, :]
    packs["upk"] = _fold_k(upk, 2)

    packs = {k: v.astype(BF16NP) for k, v in packs.items()}
    h0f = _fold128(f32(inputs["h0"])[0], 4)
    hti = np.zeros((128, 4, BL, T + 1), np.float32)
    hti[:, :, :, 0] = h0f[:, :, None]          # slot 0 = h0
    hti[16, 3, :, :] = 1.0                     # ones rail for the bias rows
    packs["hti"] = hti.astype(BF16NP)
    packs["c0f"] = _fold128(f32(inputs["c0"])[0], 4)

    # per-core x, feature-major [128, 6, L], normal and time-reversed
    xr = x[:, ::-1]
    per_core = []
    for c in range(NCORES):
        sl = x[c * BL:(c + 1) * BL]
        slr = xr[c * BL:(c + 1) * BL]
        m = dict(packs)
        m["xT"] = _fold_k(sl.transpose(2, 0, 1).reshape(D, L), NKD).astype(BF16NP)
        m["xTr"] = _fold_k(slr.transpose(2, 0, 1).reshape(D, L), NKD).astype(BF16NP)
        per_core.append(m)
    return per_core


# ------------------------------------------------------------------ program

def _build_program():
    nc = bass.Bass(trn_type="TRN2", target_bir_lowering=False, debug=False)

    dins = {}

    def din(name, shape, dt=BF16):
        dins[name] = nc.dram_tensor(name, list(shape), dt, kind="ExternalInput").ap()
        return dins[name]

    din("xT", (128, NKD, L)); din("xTr", (128, NKD, L))
    din("wx0f", (128, NKD, 3 * GP)); din("wx0b", (128, NKD, 3 * GP))
    din("wh0f", (128, NKH, 3 * GP)); din("wh0b", (128, NKH, 3 * GP))
    for s in ("1f", "1b"):
        din("wx" + s + "f", (128, NKH, 3 * GP))
        din("wx" + s + "b", (128, NKH, 3 * GP))
        din("wh" + s, (128, NKH, 3 * GP))
    din("whwf", (128, NKH, 2 * GP)); din("whwb", (128, NKH, 2 * GP))
    din("wsf", (128, NKH, F)); din("wsb", (128, NKH, F))
    din("wef", (128, NKH, F)); din("web", (128, NKH, F))
    din("upk", (128, 2, C * 256))
    din("hti", (128, 4, BL, T + 1)); din("c0f", (128, 4), dt=F32)
    out_d = nc.dram_tensor("out", [BL, T, T, C], BF16, kind="ExternalOutput").ap()

    with tile.TileContext(nc) as tc:
        _body(nc, tc, dins, out_d)
    _split_multi_waits(nc)
    return nc


def _split_multi_waits(nc, max_waits=1):
    """This container's walrus supports only one embedded sync-wait per
    instruction ("Too many sync wait commands"); hoist extra waits onto
    single-wait NoOps inserted just before, on the same engine queue.
    Sequential waiting on monotone semaphores is equivalent to the joint
    wait."""
    n = 0
    for func in nc.m.functions:
        for blk in func.blocks:
            out = []
            for inst in blk.instructions:
                si = inst.sync_info
                if si is not None and si.on_wait and len(si.on_wait) > max_waits:
                    waits = list(si.on_wait)
                    for j, w in enumerate(waits[:-max_waits]):
                        nop = mybir.InstNoOp(name=f"{inst.name}-xw{j}")
                        nop.engine = inst.engine
                        nop.sync_info = mybir.SyncInfo(on_wait=[w], on_update=[])
                        out.append(nop)
                        n += 1
                    inst.sync_info = mybir.SyncInfo(
                        on_wait=waits[-max_waits:], on_update=list(si.on_update))
                out.append(inst)
            blk.instructions = out
    return n


def _load_w(nc, pool, dram, nk, cols, tag, nsplit=1):
    """One [128, nk, cols] tile; loaded via `nsplit` DMAs along the k axis."""
    t = pool.tile([128, nk, cols], BF16, name=tag, tag=tag)
    step = (nk + nsplit - 1) // nsplit
    for a in range(0, nk, step):
        b = min(a + step, nk)
        nc.sync.dma_start(out=t[:, a:b, :], in_=dram[:, a:b, :])
    return t


def _body(nc, tc, dins, out_d):
    # Pool allocation order is the (LIFO) release order, reversed.  Base pools
    # live to the end; big transients nest inside phase windows.
    const = tc.alloc_tile_pool(name="const", bufs=1)
    ppool = tc.alloc_tile_pool(name="psum", bufs=2, space="PSUM")
    endw = tc.alloc_tile_pool(name="endw", bufs=1)        # endgame weights
    sepool = tc.alloc_tile_pool(name="se", bufs=1)        # s1/e1 (+ early ones rows)
    ht0pool = tc.alloc_tile_pool(name="ht0", bufs=1)      # f/br; reused as blend out
    trans = tc.alloc_tile_pool(name="trans", bufs=1)      # released end of phase E
    ht1pool = tc.alloc_tile_pool(name="ht1", bufs=1)      # f/b/br; released end of E
    xtpool = tc.alloc_tile_pool(name="xtilde", bufs=1)    # x~ slots shared by L0/L1
    wh1pool = tc.alloc_tile_pool(name="wh1", bufs=1)      # released end of D
    ht0tmp = tc.alloc_tile_pool(name="ht0tmp", bufs=1)    # b/fr; released end of C

    ident = const.tile([128, 128], BF16)
    make_identity(nc, ident)
    c0sb = const.tile([128, 4], F32)
    nc.sync.dma_start(out=c0sb, in_=dins["c0f"])
    # Engine APs must start at a 32-aligned partition, so "ones" rows living at
    # odd partitions are written via SBUF->SBUF DMA from this partition-0 tile.
    ones_c = const.tile([1, BL, T + 1], BF16)
    nc.vector.memset(ones_c, 1.0)

    def init_ht(ht):
        # fresh-SBUF init in ONE DMA (DMA instructions only support one wait):
        # zeros + h0 at slot 0 + the ones rail for the bias rows.
        nc.sync.dma_start(out=ht, in_=dins["hti"])

    # All recurrence state tensors are allocated and initialized up front, on
    # fresh SBUF, so their init DMAs carry at most one sync wait each (the DMA
    # lowering only supports a single wait condition).
    ht0 = {}
    ht1 = {}
    ht0["f"] = ht0pool.tile([128, 4, BL, T + 1], BF16, name="ht0f", tag="ht0f")
    ht0["br"] = ht0pool.tile([128, 4, BL, T + 1], BF16, name="ht0br", tag="ht0br")
    ht0["b"] = ht0tmp.tile([128, 4, BL, T + 1], BF16, name="ht0b", tag="ht0b")
    ht0["fr"] = ht0tmp.tile([128, 4, BL, T + 1], BF16, name="ht0fr", tag="ht0fr")
    ht1["f"] = ht1pool.tile([128, 4, BL, T + 1], BF16, name="ht1f", tag="ht1f")
    ht1["b"] = ht1pool.tile([128, 4, BL, T + 1], BF16, name="ht1b", tag="ht1b")
    ht1["br"] = ht1pool.tile([128, 4, BL, T + 1], BF16, name="ht1br", tag="ht1br")

    # -------- phase A: layer-0 x_tilde (feature-major) --------
    whpool = tc.alloc_tile_pool(name="wh0", bufs=1)
    xpool = tc.alloc_tile_pool(name="xt", bufs=1)
    xt_sb = _load_w(nc, xpool, dins["xT"], NKD, L, "xt")
    wx0 = {}
    wx0["f"] = _load_w(nc, xpool, dins["wx0f"], NKD, 3 * GP, "wx0f", nsplit=3)
    xtr_sb = _load_w(nc, xpool, dins["xTr"], NKD, L, "xtr")
    wx0["b"] = _load_w(nc, xpool, dins["wx0b"], NKD, 3 * GP, "wx0b", nsplit=3)

    # recurrence-state init + phase-B weights, issued behind the phase-A loads
    for t_ in (ht0["f"], ht0["b"], ht1["f"], ht1["b"]):
        init_ht(t_)
    wh0 = {"f": _load_w(nc, whpool, dins["wh0f"], NKH, 3 * GP, "wh0f"),
           "b": _load_w(nc, whpool, dins["wh0b"], NKH, 3 * GP, "wh0b")}
    s1T = {}
    for nm in ("s", "e"):
        st = sepool.tile([128, 2, L], BF16, name=nm + "1T", tag=nm + "1T")
        nc.sync.dma_start(out=st[F - 128:F - 127, 1, :],
                          in_=ones_c.rearrange("p b t -> p (b t)")[:, 0:L])
        s1T[nm] = st
    # endgame weights (persistent; issued early so the DMA queue drains them
    # during the long Jacobi windows)
    whw = {"f": _load_w(nc, endw, dins["whwf"], NKH, 2 * GP, "whwf"),
           "b": _load_w(nc, endw, dins["whwb"], NKH, 2 * GP, "whwb")}
    wse = {}
    for nm in ("s", "e"):
        wse[nm] = {"f": _load_w(nc, endw, dins["w" + nm + "f"], NKH, F, "w" + nm + "f"),
                   "b": _load_w(nc, endw, dins["w" + nm + "b"], NKH, F, "w" + nm + "b")}
    ut = _load_w(nc, endw, dins["upk"], 2, C * 256, "upk")

    def psum_tile():
        return ppool.tile([128, 4, GP], F32, name="pz", tag="pz")

    xt0 = {}
    for s, mov in (("f", xt_sb), ("b", xtr_sb)):
        wt = wx0[s]
        store = xtpool.tile([128, NM, GP], BF16, name="xt0" + s, tag="xt" + s)
        for grp in range(3):
            pz = psum_tile()
            for mi in range(4):
                m = grp * 4 + mi
                for k in range(NKD):
                    nc.tensor.matmul(pz[:, mi, :], wt[:, k, m * 128:(m + 1) * 128],
                                     mov[:, k, :], start=(k == 0), stop=(k == NKD - 1))
            nc.scalar.copy(store[:, grp * 4:(grp + 1) * 4, :], pz)
        xt0[s] = store
    xpool.release()

    def lstm_jacobi_pair(streams):
        """Iterate both directions' LSTM fixed points together so the two
        streams' matmuls, activations and scans overlap across engines.
        streams = [(wh_tile, xs, ht), ...]; ht is [128, 4, 2, 257],
        pre-initialized (slot 0 = h0, ones rail at [16, 3], zeros)."""
        for _ in range(K_ITERS):
            for si, (wh_t, xs, ht) in enumerate(streams):
                ss = str(si)
                I = trans.tile([128, 4, BL, T], BF16, name="I" + ss, tag="I" + ss)
                Gt = trans.tile([128, 4, BL, T], BF16, name="Gt" + ss, tag="Gt" + ss)
                O = trans.tile([128, 4, BL, T], BF16, name="O" + ss, tag="O" + ss)
                for g, (dst, fn) in enumerate(((I, AF.Sigmoid), (Gt, AF.Tanh),
                                               (O, AF.Sigmoid))):
                    pz = psum_tile()
                    for mi in range(4):
                        m = g * 4 + mi
                        nc.tensor.matmul(pz[:, mi, :], ident, xs[:, m, :],
                                         start=True, stop=False)
                        for k in range(NKH):
                            nc.tensor.matmul(pz[:, mi, :],
                                             wh_t[:, k, m * 128:(m + 1) * 128],
                                             ht[:, k, :, 0:T],
                                             start=False, stop=(k == NKH - 1))
                    nc.scalar.activation(dst, pz.rearrange("p m (b t) -> p m b t", b=BL), fn)
                nc.vector.tensor_mul(Gt, I, Gt)                     # b_t = i * g
                nc.vector.tensor_scalar(out=I, in0=I, scalar1=-1.0, scalar2=1.0,
                                        op0=ALU.mult, op1=ALU.add)
                for k in range(4):                                   # c scan per (chunk, b)
                    for b in range(BL):
                        # in-place over Gt: the scan reads data1[t] before
                        # writing out[t], so out may alias data1
                        nc.vector.tensor_tensor_scan(
                            out=Gt[:, k, b, :], data0=I[:, k, b, :], data1=Gt[:, k, b, :],
                            initial=c0sb[:, k:k + 1],
                            op0=ALU.mult, op1=ALU.add)
                nc.scalar.activation(Gt, Gt, AF.Tanh)
                nc.vector.tensor_mul(ht[:, 0:3, :, 1:T + 1], Gt[:, 0:3], O[:, 0:3])
                nc.vector.tensor_mul(ht[0:16, 3, :, 1:T + 1], Gt[0:16, 3], O[0:16, 3])

    # -------- phase B: layer-0 recurrences (both directions interleaved) -----
    lstm_jacobi_pair([(wh0["f"], xt0["f"], ht0["f"]), (wh0["b"], xt0["b"], ht0["b"])])
    whpool.release()

    # reversed-time copies (the ones rail at [16, 3] copies over too)
    nc.vector.tensor_copy(ht0["fr"][:, :, :, 1:T + 1], ht0["f"][:, :, :, T:0:-1])
    nc.vector.tensor_copy(ht0["br"][:, :, :, 1:T + 1], ht0["b"][:, :, :, T:0:-1])

    # -------- phase C: layer-1 x_tilde --------
    wx1pool = tc.alloc_tile_pool(name="wx1", bufs=1)
    wx1 = {}
    for s in ("f", "b"):
        wx1[s] = (_load_w(nc, wx1pool, dins["wx1" + s + "f"], NKH, 3 * GP, "wx1" + s + "f"),
                  _load_w(nc, wx1pool, dins["wx1" + s + "b"], NKH, 3 * GP, "wx1" + s + "b"))
    wh1 = {"f": _load_w(nc, wh1pool, dins["wh1f"], NKH, 3 * GP, "wh1f"),
           "b": _load_w(nc, wh1pool, dins["wh1b"], NKH, 3 * GP, "wh1b")}

    xt1 = {}
    for s, (hfmov, hbmov) in (("f", (ht0["f"], ht0["br"])), ("b", (ht0["fr"], ht0["b"]))):
        wtf, wtb = wx1[s]
        store = xtpool.tile([128, NM, GP], BF16, name="xt1" + s, tag="xt" + s)
        pairs = [(wtf, hfmov, k) for k in range(NKH)] + [(wtb, hbmov, k) for k in range(NKH)]
        for grp in range(3):
            pz = psum_tile()
            for mi in range(4):
                m = grp * 4 + mi
                for pi, (wt, mov, k) in enumerate(pairs):
                    nc.tensor.matmul(pz[:, mi, :], wt[:, k, m * 128:(m + 1) * 128],
                                     mov[:, k, :, 1:T + 1],
                                     start=(pi == 0), stop=(pi == 7))
            nc.scalar.copy(store[:, grp * 4:(grp + 1) * 4, :], pz)
        xt1[s] = store
    wx1pool.release()
    ht0tmp.release()

    # -------- phase D: layer-1 recurrences (both directions interleaved) -----
    lstm_jacobi_pair([(wh1["f"], xt1["f"], ht1["f"]), (wh1["b"], xt1["b"], ht1["b"])])
    wh1pool.release()
    nc.vector.tensor_copy(ht1["br"][:, :, :, 1:T + 1], ht1["b"][:, :, :, T:0:-1])
    xtpool.release()

    # -------- phase E: highway gate + blend (in place over ht0 f/br slots) ----
    outT = {}
    pairs = [(whw["f"], ht1["f"], k) for k in range(NKH)] + \
            [(whw["b"], ht1["br"], k) for k in range(NKH)]
    for half, (h1, h0) in (("f", (ht1["f"], ht0["f"])), ("b", (ht1["br"], ht0["br"]))):
        pz = psum_tile()
        for mi in range(4):
            m = (0 if half == "f" else 4) + mi
            for pi, (wt, mov, k) in enumerate(pairs):
                nc.tensor.matmul(pz[:, mi, :], wt[:, k, m * 128:(m + 1) * 128],
                                 mov[:, k, :, 1:T + 1],
                                 start=(pi == 0), stop=(pi == 7))
        gate = trans.tile([128, 4, BL, T], BF16, name="gate", tag="I0")
        nc.scalar.activation(gate, pz.rearrange("p m (b t) -> p m b t", b=BL), AF.Sigmoid)
        tmp = trans.tile([128, 4, BL, T], BF16, name="tmpb", tag="Gt0")
        hsl = h0[:, :, :, 1:T + 1]
        nc.vector.tensor_sub(tmp, h1[:, :, :, 1:T + 1], hsl)
        nc.vector.tensor_mul(tmp, gate, tmp)
        # the final write skips partition 16 of chunk 3 so the ones rail from
        # the init image survives for the projection bias rows
        nc.vector.tensor_add(hsl[:, 0:3], hsl[:, 0:3], tmp[:, 0:3])
        nc.vector.tensor_add(hsl[0:16, 3], hsl[0:16, 3], tmp[0:16, 3])
        outT[half] = h0
    ht1pool.release()
    trans.release()

    # -------- phase F: s/e projections --------
    for nm in ("s", "e"):
        wf, wb = wse[nm]["f"], wse[nm]["b"]
        st = s1T[nm]
        prs = [(wf, outT["f"], k) for k in range(NKH)] + [(wb, outT["b"], k) for k in range(NKH)]
        pz = psum_tile()
        for mi, (ma, mb) in enumerate(((0, 128), (128, F))):
            for pi, (wt, mov, k) in enumerate(prs):
                nc.tensor.matmul(pz[0:mb - ma, mi, :], wt[:, k, ma:mb],
                                 mov[:, k, :, 1:T + 1],
                                 start=(pi == 0), stop=(pi == 7))
        nc.scalar.copy(st[:, 0, :], pz[:, 0, :])
        nc.scalar.copy(st[0:F - 128, 1, :], pz[0:F - 128, 1, :])

    # -------- phase G: biaffine part 1: tmp[(c,j), (b,t)] --------
    biapool = tc.alloc_tile_pool(name="bia", bufs=1)
    smov = [s1T["s"][:, 0, :], s1T["s"][0:F + 1 - 128, 1, :]]
    ut_t = [ut[:, 0, :], ut[0:F + 1 - 128, 1, :]]
    tmpT = biapool.tile([128, 16, GP], BF16, name="tmpT", tag="tmpT")
    for grp in range(4):
        pz = psum_tile()
        for mi in range(4):
            m = grp * 4 + mi
            for k in range(2):
                nc.tensor.matmul(pz[:, mi, :], ut_t[k][:, m * 128:(m + 1) * 128],
                                 smov[k], start=(k == 0), stop=(k == 1))
        nc.scalar.copy(tmpT[:, grp * 4:(grp + 1) * 4, :], pz)

    # -------- phase H: biaffine part 2 + output assembly --------
    emov0 = s1T["e"][:, 0, :].rearrange("p (b t) -> p b t", b=BL)
    emov1 = s1T["e"][0:F + 1 - 128, 1, :].rearrange("p (b t) -> p b t", b=BL)
    ssbpool = tc.alloc_tile_pool(name="osb", bufs=2)
    for bi in range(BL):
        for xt_i in range(2):
            osb = ssbpool.tile([128, T, C], BF16, name="osb", tag="osb")
            pz = psum_tile()
            for c in range(C):
                xsl = slice(bi * T + xt_i * 128, bi * T + xt_i * 128 + 128)
                po = pz[:, c // 2, (c % 2) * T:(c % 2) * T + T]
                nc.tensor.matmul(po, tmpT[:, 2 * c, xsl], emov0[:, bi, :],
                                 start=True, stop=False)
                nc.tensor.matmul(po, tmpT[0:F + 1 - 128, 2 * c + 1, xsl],
                                 emov1[:, bi, :], start=False, stop=True)
            # one merged copy per block: psum [128, 4, 2, 256] -> osb [t, c]
            eng = nc.vector if (bi * 2 + xt_i) % 2 == 0 else nc.scalar
            if eng is nc.vector:
                nc.vector.tensor_copy(
                    osb.rearrange("p t (chi clo) -> p chi clo t", clo=2),
                    pz.rearrange("p m (clo t) -> p m clo t", clo=2))
            else:
                nc.scalar.copy(
                    osb.rearrange("p t (chi clo) -> p chi clo t", clo=2),
                    pz.rearrange("p m (clo t) -> p m clo t", clo=2))
            nc.sync.dma_start(out=out_d[bi, xt_i * 128:(xt_i + 1) * 128, :, :], in_=osb)
    ssbpool.release()
    biapool.release()
    ht0pool.release()
    sepool.release()
    endw.release()
    ppool.release()
    const.release()


# ------------------------------------------------------------------ entry point

TRACE = False          # set True (from test harnesses) to capture an NTFF profile
LAST_RESULT = None     # BassKernelResults of the most recent run


def kernel(**inputs) -> np.ndarray:
    global LAST_RESULT
    if "nc" not in _CACHE:
        _CACHE["nc"] = _build_program()
    nc = _CACHE["nc"]
    in_maps = _pack_inputs(inputs)
    try:
        res = run_bass_kernel_spmd(nc, in_maps, core_ids=list(range(NCORES)),
                                   trace=TRACE)
    except ModuleNotFoundError:
        # no NTFF profile hook in this container; run without tracing
        res = run_bass_kernel_spmd(nc, in_maps, core_ids=list(range(NCORES)))
    LAST_RESULT = res
    out = np.concatenate([np.asarray(res.results[c]["out"]) for c in range(NCORES)],
                         axis=0)
    return np.ascontiguousarray(out.astype(np.float32))


if __name__ == "__main__":
    raise SystemExit("use test.py")


# revision 22
# speedup vs baseline: 1.0214x; 1.0214x over previous
"""Biaffine NER model (2-layer BiLSTM + highway + biaffine) on 8 Trainium2 cores.

Strategy:
  - Data-parallel over batch: each of the 8 cores handles B_loc=2 of the 16
    batch elements, full model, no collectives.
  - The LSTM recurrences are solved by fixed-point (Jacobi) iteration:
      H^{k+1} = LSTMCell(x_tilde + shift(H^k) @ W_h)
    Each iteration is fully parallel over time (big matmuls, M = B_loc*T = 512
    rows), and the cell-state recurrence c_t = a_t*c_{t-1} + b_t is computed
    with the hardware tensor_tensor_scan. The map contracts by ~4x per
    iteration; K_ITERS=5 sits at ~9.5e-3 rel absmax vs the 2e-2 gate.
  - Everything on-chip is kept "transposed" (feature-major, [128-partition
    folds, (b, t) free]) so matmuls, activations and scans all operate on
    full-width tiles.
  - All contraction dims are zero-padded to multiples of 128 host-side so
    every matmul uses uniform full-128 K-tiles (padding rows are zero on both
    the stationary and moving side).
  - Elementwise gate math runs in bf16 (DVE 2x/4x perf modes).
  - Biases ride as an extra contraction row (ones rail in the moving operand,
    bias row in the stationary operand).
  - Output is DMA'd as bf16 and upcast host-side (halves the output-write
    tail; adds <4e-4 rel err).
"""

import sys

sys.path.insert(0, "/opt/trn_rl_repo")

import ml_dtypes
import numpy as np

import concourse.bass as bass
import concourse.mybir as mybir
import concourse.tile as tile
from concourse.bass_utils import run_bass_kernel_spmd
from concourse.masks import make_identity

F32 = mybir.dt.float32
BF16 = mybir.dt.bfloat16
FP8 = mybir.dt.float8e4
BF16NP = ml_dtypes.bfloat16
F8NP = ml_dtypes.float8_e4m3
AF = mybir.ActivationFunctionType
ALU = mybir.AluOpType
DR = mybir.MatmulPerfMode.DoubleRow
W8SCALE = 128.0           # fp8 weight pre-scale (e4m3 max-normal is 240)

B, T, D = 16, 256, 768
H, H2, G = 400, 800, 1200
F, C = 150, 8
NCORES = 8
BL = B // NCORES          # 2 batch elements per core
L = BL * T                # 512 (b, t) rows per core
GP = 512                  # per-gate padded stride (3*GP = 1536, 12 M-tiles)
NM = 12                   # M-tiles of the padded gate dim
NKH = 4                   # K-tiles of the padded [H+1->512] contraction
NKD = 6                   # K-tiles of D=768
K_ITERS = 5

_CACHE = {}


# ------------------------------------------------------------------ host packing

def _pack_gate_cols(w):
    """[K, 3H] -> [K, 3*GP] with each gate's 400 cols padded to 512."""
    k = w.shape[0]
    out = np.zeros((k, 3 * GP), np.float32)
    for g in range(3):
        out[:, g * GP:g * GP + H] = w[:, g * H:(g + 1) * H]
    return out


def _with_bias_row(w, bias):
    """Append one row (the bias, packed like w's columns) to w."""
    return np.concatenate([w, bias[None, :]], 0)


def _fold_k(w, nk):
    """[K<=128*nk, C] -> [128, nk, C] zero-padded row fold (row r -> [r%128, r//128])."""
    k, c = w.shape
    out = np.zeros((128 * nk, c), np.float32)
    out[:k] = w
    return np.ascontiguousarray(out.reshape(nk, 128, c).transpose(1, 0, 2))


def _fold128(v, nchunk):
    """[n] -> [128, nchunk] column-major fold (unit u -> [u%128, u//128])."""
    out = np.zeros((128, nchunk), np.float32)
    n = len(v)
    for m in range(nchunk):
        seg = v[m * 128:min((m + 1) * 128, n)]
        out[:len(seg), m] = seg
    return out


def _pack_inputs(inputs):
    """Pack weights into the DRAM layouts the program expects (shared by all cores)."""
    f32 = lambda a: np.ascontiguousarray(np.asarray(a, np.float32))
    x = f32(inputs["x"])
    z = np.zeros((3 * GP,), np.float32)

    packs = {}
    fp8packs = {}

    def _fp8_pairs(whfold):
        """[128, 4, C] bf-side fold -> [128, 2(pair), 2(slot), C] fp8, x128."""
        w8 = np.clip(whfold * W8SCALE, -240.0, 240.0).astype(F8NP)
        return np.ascontiguousarray(w8.reshape(128, 2, 2, -1))

    # layer 0: W [D+H, 3H].  The gate bias rides as the last row of wh (it is
    # re-added every Jacobi iteration through the ones slot of ht).
    for nm, wn, bn in (("0f", "W_f0", "b_f0"), ("0b", "W_b0", "b_b0")):
        W = f32(inputs[wn]); bias = _pack_gate_cols(f32(inputs[bn])[None, :])[0]
        packs["wx" + nm] = _fold_k(_pack_gate_cols(W[:D]), NKD)
        wh = _fold_k(_with_bias_row(_pack_gate_cols(W[D:]), bias), NKH)
        packs["wh" + nm] = wh
        fp8packs["wh" + nm + "8"] = _fp8_pairs(wh)
    # layer 1: W [2H+H, 3H]; the input half splits into hf/hb parts (both with
    # zero bias rows -- the bias lives only in wh).
    for nm, wn, bn in (("1f", "W_f1", "b_f1"), ("1b", "W_b1", "b_b1")):
        W = f32(inputs[wn]); bias = _pack_gate_cols(f32(inputs[bn])[None, :])[0]
        packs["wx" + nm + "f"] = _fold_k(_with_bias_row(_pack_gate_cols(W[:H]), z), NKH)
        packs["wx" + nm + "b"] = _fold_k(_with_bias_row(_pack_gate_cols(W[H:H2]), z), NKH)
        wh = _fold_k(_with_bias_row(_pack_gate_cols(W[H2:]), bias), NKH)
        packs["wh" + nm] = wh
        fp8packs["wh" + nm + "8"] = _fp8_pairs(wh)

    # highway: W_hw [2H, 2H]; M packed as [f-half pad 512 | b-half pad 512]
    Whw = f32(inputs["W_hw"]); bhw = f32(inputs["b_hw"])

    def _pack_hw_cols(w):
        k = w.shape[0]
        out = np.zeros((k, 2 * GP), np.float32)
        out[:, 0:H] = w[:, 0:H]
        out[:, GP:GP + H] = w[:, H:H2]
        return out

    zh = np.zeros((2 * GP,), np.float32)
    packs["whwf"] = _fold_k(_with_bias_row(_pack_hw_cols(Whw[:H]), _pack_hw_cols(bhw[None, :])[0]), NKH)
    packs["whwb"] = _fold_k(_with_bias_row(_pack_hw_cols(Whw[H:]), zh), NKH)

    # projections: Ws/We [2H, F]
    for nm, wn, bn in (("s", "W_s", "b_s"), ("e", "W_e", "b_e")):
        W = f32(inputs[wn]); bias = f32(inputs[bn])
        packs["w" + nm + "f"] = _fold_k(_with_bias_row(W[:H], bias), NKH)
        packs["w" + nm + "b"] = _fold_k(_with_bias_row(W[H:], np.zeros((F,), np.float32)), NKH)

    # biaffine U [F+1, C, F+1] -> [F+1, C*256] (each c padded 151->256)
    U = f32(inputs["U"])
    upk = np.zeros((F + 1, C * 256), np.float32)
    for c in range(C):
        upk[:, c * 256:c * 256 + F + 1] = U[:, c, :]
    packs["upk"] = _fold_k(upk, 2)

    packs = {k: v.astype(BF16NP) for k, v in packs.items()}
    packs.update(fp8packs)
    h0f = _fold128(f32(inputs["h0"])[0], 4)
    hti = np.zeros((128, 4, BL, T + 1), np.float32)
    hti[:, :, :, 0] = h0f[:, :, None]          # slot 0 = h0
    hti[16, 3, :, :] = 1.0                     # ones rail for the bias rows
    packs["hti"] = hti.astype(BF16NP)
    packs["hti8"] = hti.astype(F8NP)
    packs["c0f"] = _fold128(f32(inputs["c0"])[0], 4)

    # per-core x, feature-major [128, 6, L]
    per_core = []
    for c in range(NCORES):
        sl = x[c * BL:(c + 1) * BL]
        m = dict(packs)
        m["xT"] = _fold_k(sl.transpose(2, 0, 1).reshape(D, L), NKD).astype(BF16NP)
        per_core.append(m)
    return per_core


# ------------------------------------------------------------------ program

def _build_program():
    nc = bass.Bass(trn_type="TRN2", target_bir_lowering=False, debug=False)

    dins = {}

    def din(name, shape, dt=BF16):
        dins[name] = nc.dram_tensor(name, list(shape), dt, kind="ExternalInput").ap()
        return dins[name]

    din("xT", (128, NKD, L))
    din("wx0f", (128, NKD, 3 * GP)); din("wx0b", (128, NKD, 3 * GP))
    din("wh0f", (128, NKH, 3 * GP)); din("wh0b", (128, NKH, 3 * GP))
    for s in ("1f", "1b"):
        din("wx" + s + "f", (128, NKH, 3 * GP))
        din("wx" + s + "b", (128, NKH, 3 * GP))
        din("wh" + s, (128, NKH, 3 * GP))
    for s in ("0f", "0b", "1f", "1b"):
        din("wh" + s + "8", (128, 2, 2, 3 * GP), dt=FP8)
    din("whwf", (128, NKH, 2 * GP)); din("whwb", (128, NKH, 2 * GP))
    din("wsf", (128, NKH, F)); din("wsb", (128, NKH, F))
    din("wef", (128, NKH, F)); din("web", (128, NKH, F))
    din("upk", (128, 2, C * 256))
    din("hti", (128, 4, BL, T + 1)); din("hti8", (128, 4, BL, T + 1), dt=FP8)
    din("c0f", (128, 4), dt=F32)
    out_d = nc.dram_tensor("out", [BL, T, T, C], BF16, kind="ExternalOutput").ap()

    with tile.TileContext(nc) as tc:
        _body(nc, tc, dins, out_d)
    _split_multi_waits(nc)
    return nc


def _split_multi_waits(nc, max_waits=1):
    """This container's walrus supports only one embedded sync-wait per
    instruction ("Too many sync wait commands"); hoist extra waits onto
    single-wait NoOps inserted just before, on the same engine queue.
    Sequential waiting on monotone semaphores is equivalent to the joint
    wait."""
    n = 0
    for func in nc.m.functions:
        for blk in func.blocks:
            out = []
            for inst in blk.instructions:
                si = inst.sync_info
                if si is not None and si.on_wait and len(si.on_wait) > max_waits:
                    waits = list(si.on_wait)
                    for j, w in enumerate(waits[:-max_waits]):
                        nop = mybir.InstNoOp(name=f"{inst.name}-xw{j}")
                        nop.engine = inst.engine
                        nop.sync_info = mybir.SyncInfo(on_wait=[w], on_update=[])
                        out.append(nop)
                        n += 1
                    inst.sync_info = mybir.SyncInfo(
                        on_wait=waits[-max_waits:], on_update=list(si.on_update))
                out.append(inst)
            blk.instructions = out
    return n


def _load_w(nc, pool, dram, nk, cols, tag, nsplit=1):
    """One [128, nk, cols] tile; loaded via `nsplit` DMAs along the k axis."""
    t = pool.tile([128, nk, cols], BF16, name=tag, tag=tag)
    step = (nk + nsplit - 1) // nsplit
    for a in range(0, nk, step):
        b = min(a + step, nk)
        nc.sync.dma_start(out=t[:, a:b, :], in_=dram[:, a:b, :])
    return t


def _body(nc, tc, dins, out_d):
    # Pool allocation order is the (LIFO) release order, reversed.  Base pools
    # live to the end; big transients nest inside phase windows.
    const = tc.alloc_tile_pool(name="const", bufs=1)
    ppool = tc.alloc_tile_pool(name="psum", bufs=2, space="PSUM")
    endw = tc.alloc_tile_pool(name="endw", bufs=1)        # endgame weights
    sepool = tc.alloc_tile_pool(name="se", bufs=1)        # s1/e1 (+ early ones rows)
    ht0pool = tc.alloc_tile_pool(name="ht0", bufs=1)      # f/br; reused as blend out
    trans = tc.alloc_tile_pool(name="trans", bufs=1)      # released end of phase E
    ht1pool = tc.alloc_tile_pool(name="ht1", bufs=1)      # f/b/br; released end of E
    xtpool = tc.alloc_tile_pool(name="xtilde", bufs=1)    # x~ slots shared by L0/L1
    wh1pool = tc.alloc_tile_pool(name="wh1", bufs=1)      # released end of D
    ht0tmp = tc.alloc_tile_pool(name="ht0tmp", bufs=1)    # b/fr; released end of C

    ident = const.tile([128, 128], BF16)
    make_identity(nc, ident)
    # scaled identity used to inject x~ into the fp8-scaled PSUM groups
    ident128 = const.tile([128, 128], BF16)
    make_identity(nc, ident128)
    nc.vector.tensor_scalar(out=ident128, in0=ident128, scalar1=W8SCALE,
                            scalar2=None, op0=ALU.mult)
    c0sb = const.tile([128, 4], F32)
    nc.sync.dma_start(out=c0sb, in_=dins["c0f"])
    # Engine APs must start at a 32-aligned partition, so "ones" rows living at
    # odd partitions are written via SBUF->SBUF DMA from this partition-0 tile.
    ones_c = const.tile([1, BL, T + 1], BF16)
    nc.vector.memset(ones_c, 1.0)

    def init_ht(ht):
        # fresh-SBUF init in ONE DMA (DMA instructions only support one wait):
        # zeros + h0 at slot 0 + the ones rail for the bias rows.
        nc.sync.dma_start(out=ht, in_=dins["hti"])

    # All recurrence state tensors are allocated and initialized up front, on
    # fresh SBUF, so their init DMAs carry at most one sync wait each (the DMA
    # lowering only supports a single wait condition).
    ht0 = {}
    ht1 = {}
    ht8 = {}
    ht0["f"] = ht0pool.tile([128, 4, BL, T + 1], BF16, name="ht0f", tag="ht0f")
    ht0["br"] = ht0pool.tile([128, 4, BL, T + 1], BF16, name="ht0br", tag="ht0br")
    for s in ("0f", "0b", "1f", "1b"):
        ht8[s] = ht0pool.tile([128, 4, BL, T + 1], FP8, name="ht8" + s, tag="ht8" + s)
    ht0["b"] = ht0tmp.tile([128, 4, BL, T + 1], BF16, name="ht0b", tag="ht0b")
    ht0["fr"] = ht0tmp.tile([128, 4, BL, T + 1], BF16, name="ht0fr", tag="ht0fr")
    ht1["f"] = ht1pool.tile([128, 4, BL, T + 1], BF16, name="ht1f", tag="ht1f")
    ht1["b"] = ht1pool.tile([128, 4, BL, T + 1], BF16, name="ht1b", tag="ht1b")
    ht1["br"] = ht1pool.tile([128, 4, BL, T + 1], BF16, name="ht1br", tag="ht1br")

    # -------- phase A: layer-0 x_tilde (feature-major) --------
    whpool = tc.alloc_tile_pool(name="wh0", bufs=1)
    xpool = tc.alloc_tile_pool(name="xt", bufs=1)
    xt_sb = _load_w(nc, xpool, dins["xT"], NKD, L, "xt")
    # time-reversed view of the same tile for the backward stream
    xt_rev = xt_sb.rearrange("p k (b t) -> p k b t", b=BL)[:, :, :, ::-1]

    # recurrence-state init + phase-B weights, issued behind the phase-A loads
    wh0_8 = {"f": whpool.tile([128, 2, 2, 3 * GP], FP8, name="wh0f8", tag="wh0f8"),
             "b": whpool.tile([128, 2, 2, 3 * GP], FP8, name="wh0b8", tag="wh0b8")}
    nc.sync.dma_start(out=wh0_8["f"], in_=dins["wh0f8"])
    for t_ in (ht0["f"], ht0["b"], ht1["f"], ht1["b"]):
        init_ht(t_)
    for s in ("0f", "0b", "1f", "1b"):
        nc.sync.dma_start(out=ht8[s], in_=dins["hti8"])
    nc.sync.dma_start(out=wh0_8["b"], in_=dins["wh0b8"])
    wh0 = {"f": _load_w(nc, whpool, dins["wh0f"], NKH, 3 * GP, "wh0f"),
           "b": _load_w(nc, whpool, dins["wh0b"], NKH, 3 * GP, "wh0b")}
    s1T = {}
    for nm in ("s", "e"):
        st = sepool.tile([128, 2, L], BF16, name=nm + "1T", tag=nm + "1T")
        nc.sync.dma_start(out=st[F - 128:F - 127, 1, :],
                          in_=ones_c.rearrange("p b t -> p (b t)")[:, 0:L])
        s1T[nm] = st
    # highway weights (persistent; issued early so the DMA queue drains them
    # during the long Jacobi windows)
    whw = {"f": _load_w(nc, endw, dins["whwf"], NKH, 2 * GP, "whwf"),
           "b": _load_w(nc, endw, dins["whwb"], NKH, 2 * GP, "whwb")}

    def psum_tile():
        return ppool.tile([128, 4, GP], F32, name="pz", tag="pz")

    xt0 = {}
    for s in ("f", "b"):
        # both directions share one weight buffer (tag wx0): the b-dir load
        # starts as soon as the f-dir matmuls finish reading it
        wt = _load_w(nc, xpool, dins["wx0" + s], NKD, 3 * GP, "wx0", nsplit=3)
        store = xtpool.tile([128, NM, GP], BF16, name="xt0" + s, tag="xt" + s)
        for grp in range(3):
            pz = psum_tile()
            for mi in range(4):
                m = grp * 4 + mi
                for k in range(NKD):
                    mov = xt_sb[:, k, :] if s == "f" else xt_rev[:, k, :, :]
                    nc.tensor.matmul(pz[:, mi, :], wt[:, k, m * 128:(m + 1) * 128],
                                     mov, start=(k == 0), stop=(k == NKD - 1))
            nc.scalar.copy(store[:, grp * 4:(grp + 1) * 4, :], pz)
        xt0[s] = store
    xpool.release()

    def lstm_jacobi_pair(streams):
        """Iterate both directions' LSTM fixed points together so the two
        streams' matmuls, activations and scans overlap across engines.
        streams = [(wh_tile_bf16, wh_pairs_fp8, xs, ht_bf16, ht_fp8), ...];
        ht is [128, 4, 2, 257], pre-initialized (slot 0 = h0, ones rail at
        [16, 3], zeros).

        Iterations 0..K-2 run the recurrence matmuls in fp8 DoubleRow mode
        (2 K-tiles per instruction at 0.5 cycles/row) against the fp8 h-state;
        weights are pre-scaled by W8SCALE and the activation divides it back
        out.  The x~ injection stays bf16 (its accuracy persists into the
        fixed point).  The last iteration runs fully in bf16: the fp8
        quantization noise of earlier iterates contracts by ~4x per iteration,
        so only bf16-level noise survives in the final h."""
        for it in range(K_ITERS):
            fp8 = it < K_ITERS - 1
            for si, (wh_t, wh_p8, xs, ht, h8) in enumerate(streams):
                ss = str(si)
                mov = h8 if fp8 else ht
                # the last fp8 iteration feeds the bf16 one: write bf16 h
                wout = ht if it >= K_ITERS - 2 else h8
                I = trans.tile([128, 4, BL, T], BF16, name="I" + ss, tag="I" + ss)
                Gt = trans.tile([128, 4, BL, T], BF16, name="Gt" + ss, tag="Gt" + ss)
                O = trans.tile([128, 4, BL, T], BF16, name="O" + ss, tag="O" + ss)
                for g, (dst, fn) in enumerate(((I, AF.Sigmoid), (Gt, AF.Tanh),
                                               (O, AF.Sigmoid))):
                    pz = psum_tile()
                    for mi in range(4):
                        m = g * 4 + mi
                        nc.tensor.matmul(pz[:, mi, :], ident128 if fp8 else ident,
                                         xs[:, m, :], start=True, stop=False)
                        if fp8:
                            for pair in range(2):
                                nc.tensor.matmul(
                                    pz[:, mi, :],
                                    wh_p8[:, pair, :, m * 128:(m + 1) * 128],
                                    mov[:, 2 * pair:2 * pair + 2, :, 0:T],
                                    start=False, stop=(pair == 1), perf_mode=DR)
                        else:
                            for k in range(NKH):
                                nc.tensor.matmul(pz[:, mi, :],
                                                 wh_t[:, k, m * 128:(m + 1) * 128],
                                                 mov[:, k, :, 0:T],
                                                 start=False, stop=(k == NKH - 1))
                    nc.scalar.activation(dst, pz.rearrange("p m (b t) -> p m b t", b=BL),
                                         fn, scale=(1.0 / W8SCALE) if fp8 else 1.0)
                nc.vector.tensor_mul(Gt, I, Gt)                     # b_t = i * g
                nc.vector.tensor_scalar(out=I, in0=I, scalar1=-1.0, scalar2=1.0,
                                        op0=ALU.mult, op1=ALU.add)
                for k in range(4):                                   # c scan per (chunk, b)
                    for b in range(BL):
                        # in-place over Gt: the scan reads data1[t] before
                        # writing out[t], so out may alias data1
                        nc.vector.tensor_tensor_scan(
                            out=Gt[:, k, b, :], data0=I[:, k, b, :], data1=Gt[:, k, b, :],
                            initial=c0sb[:, k:k + 1],
                            op0=ALU.mult, op1=ALU.add)
                nc.scalar.activation(Gt, Gt, AF.Tanh)
                nc.vector.tensor_mul(wout[:, 0:3, :, 1:T + 1], Gt[:, 0:3], O[:, 0:3])
                nc.vector.tensor_mul(wout[0:16, 3, :, 1:T + 1], Gt[0:16, 3], O[0:16, 3])

    # -------- phase B: layer-0 recurrences (both directions interleaved) -----
    lstm_jacobi_pair([(wh0["f"], wh0_8["f"], xt0["f"], ht0["f"], ht8["0f"]),
                      (wh0["b"], wh0_8["b"], xt0["b"], ht0["b"], ht8["0b"])])
    whpool.release()

    # reversed-time copies (the ones rail at [16, 3] copies over too)
    nc.vector.tensor_copy(ht0["fr"][:, :, :, 1:T + 1], ht0["f"][:, :, :, T:0:-1])
    nc.vector.tensor_copy(ht0["br"][:, :, :, 1:T + 1], ht0["b"][:, :, :, T:0:-1])

    # -------- phase C: layer-1 x_tilde --------
    wx1pool = tc.alloc_tile_pool(name="wx1", bufs=1)
    wh1_8 = {"f": wh1pool.tile([128, 2, 2, 3 * GP], FP8, name="wh1f8", tag="wh1f8"),
             "b": wh1pool.tile([128, 2, 2, 3 * GP], FP8, name="wh1b8", tag="wh1b8")}
    nc.sync.dma_start(out=wh1_8["f"], in_=dins["wh1f8"])
    nc.sync.dma_start(out=wh1_8["b"], in_=dins["wh1b8"])
    wx1 = {}
    for s in ("f", "b"):
        wx1[s] = (_load_w(nc, wx1pool, dins["wx1" + s + "f"], NKH, 3 * GP, "wx1" + s + "f"),
                  _load_w(nc, wx1pool, dins["wx1" + s + "b"], NKH, 3 * GP, "wx1" + s + "b"))
    wh1 = {"f": _load_w(nc, wh1pool, dins["wh1f"], NKH, 3 * GP, "wh1f"),
           "b": _load_w(nc, wh1pool, dins["wh1b"], NKH, 3 * GP, "wh1b")}

    xt1 = {}
    for s, (hfmov, hbmov) in (("f", (ht0["f"], ht0["br"])), ("b", (ht0["fr"], ht0["b"]))):
        wtf, wtb = wx1[s]
        store = xtpool.tile([128, NM, GP], BF16, name="xt1" + s, tag="xt" + s)
        pairs = [(wtf, hfmov, k) for k in range(NKH)] + [(wtb, hbmov, k) for k in range(NKH)]
        for grp in range(3):
            pz = psum_tile()
            for mi in range(4):
                m = grp * 4 + mi
                for pi, (wt, mov, k) in enumerate(pairs):
                    nc.tensor.matmul(pz[:, mi, :], wt[:, k, m * 128:(m + 1) * 128],
                                     mov[:, k, :, 1:T + 1],
                                     start=(pi == 0), stop=(pi == 7))
            nc.scalar.copy(store[:, grp * 4:(grp + 1) * 4, :], pz)
        xt1[s] = store
    wx1pool.release()
    ht0tmp.release()

    # -------- phase D: layer-1 recurrences (both directions interleaved) -----
    lstm_jacobi_pair([(wh1["f"], wh1_8["f"], xt1["f"], ht1["f"], ht8["1f"]),
                      (wh1["b"], wh1_8["b"], xt1["b"], ht1["b"], ht8["1b"])])
    wh1pool.release()
    nc.vector.tensor_copy(ht1["br"][:, :, :, 1:T + 1], ht1["b"][:, :, :, T:0:-1])
    xtpool.release()

    # -------- phase E: highway gate + blend (in place over ht0 f/br slots) ----
    outT = {}
    pairs = [(whw["f"], ht1["f"], k) for k in range(NKH)] + \
            [(whw["b"], ht1["br"], k) for k in range(NKH)]
    for half, (h1, h0) in (("f", (ht1["f"], ht0["f"])), ("b", (ht1["br"], ht0["br"]))):
        pz = psum_tile()
        for mi in range(4):
            m = (0 if half == "f" else 4) + mi
            for pi, (wt, mov, k) in enumerate(pairs):
                nc.tensor.matmul(pz[:, mi, :], wt[:, k, m * 128:(m + 1) * 128],
                                 mov[:, k, :, 1:T + 1],
                                 start=(pi == 0), stop=(pi == 7))
        gate = trans.tile([128, 4, BL, T], BF16, name="gate", tag="I0")
        nc.scalar.activation(gate, pz.rearrange("p m (b t) -> p m b t", b=BL), AF.Sigmoid)
        tmp = trans.tile([128, 4, BL, T], BF16, name="tmpb", tag="Gt0")
        hsl = h0[:, :, :, 1:T + 1]
        nc.vector.tensor_sub(tmp, h1[:, :, :, 1:T + 1], hsl)
        nc.vector.tensor_mul(tmp, gate, tmp)
        # the final write skips partition 16 of chunk 3 so the ones rail from
        # the init image survives for the projection bias rows
        nc.vector.tensor_add(hsl[:, 0:3], hsl[:, 0:3], tmp[:, 0:3])
        nc.vector.tensor_add(hsl[0:16, 3], hsl[0:16, 3], tmp[0:16, 3])
        outT[half] = h0
    ht1pool.release()
    trans.release()

    # -------- phase F: s/e projections --------
    latew = tc.alloc_tile_pool(name="latew", bufs=1)
    wse = {}
    for nm in ("s", "e"):
        wse[nm] = {"f": _load_w(nc, latew, dins["w" + nm + "f"], NKH, F, "w" + nm + "f"),
                   "b": _load_w(nc, latew, dins["w" + nm + "b"], NKH, F, "w" + nm + "b")}
    ut = _load_w(nc, latew, dins["upk"], 2, C * 256, "upk")
    for nm in ("s", "e"):
        wf, wb = wse[nm]["f"], wse[nm]["b"]
        st = s1T[nm]
        prs = [(wf, outT["f"], k) for k in range(NKH)] + [(wb, outT["b"], k) for k in range(NKH)]
        pz = psum_tile()
        for mi, (ma, mb) in enumerate(((0, 128), (128, F))):
            for pi, (wt, mov, k) in enumerate(prs):
                nc.tensor.matmul(pz[0:mb - ma, mi, :], wt[:, k, ma:mb],
                                 mov[:, k, :, 1:T + 1],
                                 start=(pi == 0), stop=(pi == 7))
        nc.scalar.copy(st[:, 0, :], pz[:, 0, :])
        nc.scalar.copy(st[0:F - 128, 1, :], pz[0:F - 128, 1, :])

    # -------- phase G: biaffine part 1: tmp[(c,j), (b,t)] --------
    biapool = tc.alloc_tile_pool(name="bia", bufs=1)
    smov = [s1T["s"][:, 0, :], s1T["s"][0:F + 1 - 128, 1, :]]
    ut_t = [ut[:, 0, :], ut[0:F + 1 - 128, 1, :]]
    tmpT = biapool.tile([128, 16, GP], BF16, name="tmpT", tag="tmpT")
    for grp in range(4):
        pz = psum_tile()
        for mi in range(4):
            m = grp * 4 + mi
            for k in range(2):
                nc.tensor.matmul(pz[:, mi, :], ut_t[k][:, m * 128:(m + 1) * 128],
                                 smov[k], start=(k == 0), stop=(k == 1))
        nc.scalar.copy(tmpT[:, grp * 4:(grp + 1) * 4, :], pz)

    # -------- phase H: biaffine part 2 + output assembly --------
    emov0 = s1T["e"][:, 0, :].rearrange("p (b t) -> p b t", b=BL)
    emov1 = s1T["e"][0:F + 1 - 128, 1, :].rearrange("p (b t) -> p b t", b=BL)
    ssbpool = tc.alloc_tile_pool(name="osb", bufs=2)
    for bi in range(BL):
        for xt_i in range(2):
            osb = ssbpool.tile([128, T, C], BF16, name="osb", tag="osb")
            pz = psum_tile()
            for c in range(C):
                xsl = slice(bi * T + xt_i * 128, bi * T + xt_i * 128 + 128)
                po = pz[:, c // 2, (c % 2) * T:(c % 2) * T + T]
                nc.tensor.matmul(po, tmpT[:, 2 * c, xsl], emov0[:, bi, :],
                                 start=True, stop=False)
                nc.tensor.matmul(po, tmpT[0:F + 1 - 128, 2 * c + 1, xsl],
                                 emov1[:, bi, :], start=False, stop=True)
            # one merged copy per block: psum [128, 4, 2, 256] -> osb [t, c]
            eng = nc.vector if (bi * 2 + xt_i) % 2 == 0 else nc.scalar
            if eng is nc.vector:
                nc.vector.tensor_copy(
                    osb.rearrange("p t (chi clo) -> p chi clo t", clo=2),
                    pz.rearrange("p m (clo t) -> p m clo t", clo=2))
            else:
                nc.scalar.copy(
                    osb.rearrange("p t (chi clo) -> p chi clo t", clo=2),
                    pz.rearrange("p m (clo t) -> p m clo t", clo=2))
            nc.sync.dma_start(out=out_d[bi, xt_i * 128:(xt_i + 1) * 128, :, :], in_=osb)
    ssbpool.release()
    biapool.release()
    latew.release()
    ht0pool.release()
    sepool.release()
    endw.release()
    ppool.release()
    const.release()


# ------------------------------------------------------------------ entry point

TRACE = False          # set True (from test harnesses) to capture an NTFF profile
LAST_RESULT = None     # BassKernelResults of the most recent run


def kernel(**inputs) -> np.ndarray:
    global LAST_RESULT
    if "nc" not in _CACHE:
        _CACHE["nc"] = _build_program()
    nc = _CACHE["nc"]
    in_maps = _pack_inputs(inputs)
    try:
        res = run_bass_kernel_spmd(nc, in_maps, core_ids=list(range(NCORES)),
                                   trace=TRACE)
    except ModuleNotFoundError:
        # no NTFF profile hook in this container; run without tracing
        res = run_bass_kernel_spmd(nc, in_maps, core_ids=list(range(NCORES)))
    LAST_RESULT = res
    out = np.concatenate([np.asarray(res.results[c]["out"]) for c in range(NCORES)],
                         axis=0)
    return np.ascontiguousarray(out.astype(np.float32))


if __name__ == "__main__":
    raise SystemExit("use test.py")


# revision 27
# speedup vs baseline: 1.0547x; 1.0327x over previous
"""Biaffine NER model (2-layer BiLSTM + highway + biaffine) on 8 Trainium2 cores.

Strategy:
  - Data-parallel over batch: each of the 8 cores handles B_loc=2 of the 16
    batch elements, full model, no collectives.
  - The LSTM recurrences are solved by fixed-point (Jacobi) iteration:
      H^{k+1} = LSTMCell(x_tilde + shift(H^k) @ W_h)
    Each iteration is fully parallel over time (big matmuls, M = B_loc*T = 512
    rows), and the cell-state recurrence c_t = a_t*c_{t-1} + b_t is computed
    with the hardware tensor_tensor_scan. The map contracts by ~4x per
    iteration; K_ITERS=5 sits at ~9.5e-3 rel absmax vs the 2e-2 gate.
  - Everything on-chip is kept "transposed" (feature-major, [128-partition
    folds, (b, t) free]) so matmuls, activations and scans all operate on
    full-width tiles.
  - All contraction dims are zero-padded to multiples of 128 host-side so
    every matmul uses uniform full-128 K-tiles (padding rows are zero on both
    the stationary and moving side).
  - Elementwise gate math runs in bf16 (DVE 2x/4x perf modes).
  - Biases ride as an extra contraction row (ones rail in the moving operand,
    bias row in the stationary operand).
  - Output is DMA'd as bf16 and upcast host-side (halves the output-write
    tail; adds <4e-4 rel err).
"""

import sys

sys.path.insert(0, "/opt/trn_rl_repo")

import ml_dtypes
import numpy as np

import concourse.bass as bass
import concourse.mybir as mybir
import concourse.tile as tile
from concourse.bass_utils import run_bass_kernel_spmd
from concourse.masks import make_identity

F32 = mybir.dt.float32
BF16 = mybir.dt.bfloat16
FP8 = mybir.dt.float8e4
BF16NP = ml_dtypes.bfloat16
F8NP = ml_dtypes.float8_e4m3
AF = mybir.ActivationFunctionType
ALU = mybir.AluOpType
DR = mybir.MatmulPerfMode.DoubleRow
W8SCALE = 128.0           # fp8 weight pre-scale (e4m3 max-normal is 240)

B, T, D = 16, 256, 768
H, H2, G = 400, 800, 1200
F, C = 150, 8
NCORES = 8
BL = B // NCORES          # 2 batch elements per core
L = BL * T                # 512 (b, t) rows per core
GP = 512                  # per-gate padded stride (3*GP = 1536, 12 M-tiles)
NM = 12                   # M-tiles of the padded gate dim
NKH = 4                   # K-tiles of the padded [H+1->512] contraction
NKD = 6                   # K-tiles of D=768
K_ITERS = 5

_CACHE = {}


# ------------------------------------------------------------------ host packing

def _pack_gate_cols(w):
    """[K, 3H] -> [K, 3*GP] with each gate's 400 cols padded to 512."""
    k = w.shape[0]
    out = np.zeros((k, 3 * GP), np.float32)
    for g in range(3):
        out[:, g * GP:g * GP + H] = w[:, g * H:(g + 1) * H]
    return out


def _with_bias_row(w, bias):
    """Append one row (the bias, packed like w's columns) to w."""
    return np.concatenate([w, bias[None, :]], 0)


def _fold_k(w, nk):
    """[K<=128*nk, C] -> [128, nk, C] zero-padded row fold (row r -> [r%128, r//128])."""
    k, c = w.shape
    out = np.zeros((128 * nk, c), np.float32)
    out[:k] = w
    return np.ascontiguousarray(out.reshape(nk, 128, c).transpose(1, 0, 2))


def _fold128(v, nchunk):
    """[n] -> [128, nchunk] column-major fold (unit u -> [u%128, u//128])."""
    out = np.zeros((128, nchunk), np.float32)
    n = len(v)
    for m in range(nchunk):
        seg = v[m * 128:min((m + 1) * 128, n)]
        out[:len(seg), m] = seg
    return out


def _pack_inputs(inputs):
    """Pack weights into the DRAM layouts the program expects (shared by all cores)."""
    f32 = lambda a: np.ascontiguousarray(np.asarray(a, np.float32))
    x = f32(inputs["x"])
    z = np.zeros((3 * GP,), np.float32)

    packs = {}
    fp8packs = {}

    def _fp8_pairs(whfold):
        """[128, 4, C] bf-side fold -> [128, 2(pair), 2(slot), C] fp8, x128."""
        w8 = np.clip(whfold * W8SCALE, -240.0, 240.0).astype(F8NP)
        return np.ascontiguousarray(w8.reshape(128, 2, 2, -1))

    # layer 0: W [D+H, 3H].  The gate bias rides as the last row of wh (it is
    # re-added every Jacobi iteration through the ones slot of ht).
    for nm, wn, bn in (("0f", "W_f0", "b_f0"), ("0b", "W_b0", "b_b0")):
        W = f32(inputs[wn]); bias = _pack_gate_cols(f32(inputs[bn])[None, :])[0]
        packs["wx" + nm] = _fold_k(_pack_gate_cols(W[:D]), NKD)
        wh = _fold_k(_with_bias_row(_pack_gate_cols(W[D:]), bias), NKH)
        packs["wh" + nm] = wh
        fp8packs["wh" + nm + "8"] = _fp8_pairs(wh)
    # layer 1: W [2H+H, 3H]; the input half splits into hf/hb parts (both with
    # zero bias rows -- the bias lives only in wh).
    for nm, wn, bn in (("1f", "W_f1", "b_f1"), ("1b", "W_b1", "b_b1")):
        W = f32(inputs[wn]); bias = _pack_gate_cols(f32(inputs[bn])[None, :])[0]
        packs["wx" + nm + "f"] = _fold_k(_with_bias_row(_pack_gate_cols(W[:H]), z), NKH)
        packs["wx" + nm + "b"] = _fold_k(_with_bias_row(_pack_gate_cols(W[H:H2]), z), NKH)
        wh = _fold_k(_with_bias_row(_pack_gate_cols(W[H2:]), bias), NKH)
        packs["wh" + nm] = wh
        fp8packs["wh" + nm + "8"] = _fp8_pairs(wh)

    # highway: W_hw [2H, 2H]; M packed as [f-half pad 512 | b-half pad 512]
    Whw = f32(inputs["W_hw"]); bhw = f32(inputs["b_hw"])

    def _pack_hw_cols(w):
        k = w.shape[0]
        out = np.zeros((k, 2 * GP), np.float32)
        out[:, 0:H] = w[:, 0:H]
        out[:, GP:GP + H] = w[:, H:H2]
        return out

    zh = np.zeros((2 * GP,), np.float32)
    packs["whwf"] = _fold_k(_with_bias_row(_pack_hw_cols(Whw[:H]), _pack_hw_cols(bhw[None, :])[0]), NKH)
    packs["whwb"] = _fold_k(_with_bias_row(_pack_hw_cols(Whw[H:]), zh), NKH)

    # projections: Ws/We [2H, F]
    for nm, wn, bn in (("s", "W_s", "b_s"), ("e", "W_e", "b_e")):
        W = f32(inputs[wn]); bias = f32(inputs[bn])
        packs["w" + nm + "f"] = _fold_k(_with_bias_row(W[:H], bias), NKH)
        packs["w" + nm + "b"] = _fold_k(_with_bias_row(W[H:], np.zeros((F,), np.float32)), NKH)

    # biaffine U [F+1, C, F+1] -> [F+1, C*256] (each c padded 151->256)
    U = f32(inputs["U"])
    upk = np.zeros((F + 1, C * 256), np.float32)
    for c in range(C):
        upk[:, c * 256:c * 256 + F + 1] = U[:, c, :]
    packs["upk"] = _fold_k(upk, 2)

    packs = {k: v.astype(BF16NP) for k, v in packs.items()}
    packs.update(fp8packs)
    h0f = _fold128(f32(inputs["h0"])[0], 4)
    hti = np.zeros((128, 4, BL, T + 1), np.float32)
    hti[:, :, :, 0] = h0f[:, :, None]          # slot 0 = h0
    hti[16, 3, :, :] = 1.0                     # ones rail for the bias rows
    packs["hti"] = hti.astype(BF16NP)
    packs["hti8"] = hti.astype(F8NP)
    packs["c0f"] = _fold128(f32(inputs["c0"])[0], 4)

    # per-core x, feature-major [128, 6, L]
    per_core = []
    for c in range(NCORES):
        sl = x[c * BL:(c + 1) * BL]
        m = dict(packs)
        m["xT"] = _fold_k(sl.transpose(2, 0, 1).reshape(D, L), NKD).astype(BF16NP)
        per_core.append(m)
    return per_core


# ------------------------------------------------------------------ program

def _build_program():
    nc = bass.Bass(trn_type="TRN2", target_bir_lowering=False, debug=False)

    dins = {}

    def din(name, shape, dt=BF16):
        dins[name] = nc.dram_tensor(name, list(shape), dt, kind="ExternalInput").ap()
        return dins[name]

    din("xT", (128, NKD, L))
    din("wx0f", (128, NKD, 3 * GP)); din("wx0b", (128, NKD, 3 * GP))
    din("wh0f", (128, NKH, 3 * GP)); din("wh0b", (128, NKH, 3 * GP))
    for s in ("1f", "1b"):
        din("wx" + s + "f", (128, NKH, 3 * GP))
        din("wx" + s + "b", (128, NKH, 3 * GP))
        din("wh" + s, (128, NKH, 3 * GP))
    for s in ("0f", "0b", "1f", "1b"):
        din("wh" + s + "8", (128, 2, 2, 3 * GP), dt=FP8)
    din("whwf", (128, NKH, 2 * GP)); din("whwb", (128, NKH, 2 * GP))
    din("wsf", (128, NKH, F)); din("wsb", (128, NKH, F))
    din("wef", (128, NKH, F)); din("web", (128, NKH, F))
    din("upk", (128, 2, C * 256))
    din("hti", (128, 4, BL, T + 1)); din("hti8", (128, 4, BL, T + 1), dt=FP8)
    din("c0f", (128, 4), dt=F32)
    out_d = nc.dram_tensor("out", [BL, T, T, C], BF16, kind="ExternalOutput").ap()

    with tile.TileContext(nc) as tc:
        _body(nc, tc, dins, out_d)
    _split_multi_waits(nc)
    return nc


def _split_multi_waits(nc, max_waits=1):
    """This container's walrus supports only one embedded sync-wait per
    instruction ("Too many sync wait commands"); hoist extra waits onto
    single-wait NoOps inserted just before, on the same engine queue.
    Sequential waiting on monotone semaphores is equivalent to the joint
    wait."""
    n = 0
    for func in nc.m.functions:
        for blk in func.blocks:
            out = []
            for inst in blk.instructions:
                si = inst.sync_info
                if si is not None and si.on_wait and len(si.on_wait) > max_waits:
                    waits = list(si.on_wait)
                    for j, w in enumerate(waits[:-max_waits]):
                        nop = mybir.InstNoOp(name=f"{inst.name}-xw{j}")
                        nop.engine = inst.engine
                        nop.sync_info = mybir.SyncInfo(on_wait=[w], on_update=[])
                        out.append(nop)
                        n += 1
                    inst.sync_info = mybir.SyncInfo(
                        on_wait=waits[-max_waits:], on_update=list(si.on_update))
                out.append(inst)
            blk.instructions = out
    return n


def _load_w(nc, pool, dram, nk, cols, tag, nsplit=1, eng=None):
    """One [128, nk, cols] tile; loaded via `nsplit` DMAs along the k axis."""
    t = pool.tile([128, nk, cols], BF16, name=tag, tag=tag)
    step = (nk + nsplit - 1) // nsplit
    for a in range(0, nk, step):
        b = min(a + step, nk)
        (eng or nc.sync).dma_start(out=t[:, a:b, :], in_=dram[:, a:b, :])
    return t


def _body(nc, tc, dins, out_d):
    # Pool allocation order is the (LIFO) release order, reversed.  Base pools
    # live to the end; big transients nest inside phase windows.
    const = tc.alloc_tile_pool(name="const", bufs=1)
    ppool = tc.alloc_tile_pool(name="psum", bufs=2, space="PSUM")
    endw = tc.alloc_tile_pool(name="endw", bufs=1)        # endgame weights
    sepool = tc.alloc_tile_pool(name="se", bufs=1)        # s1/e1 (+ early ones rows)
    ht0pool = tc.alloc_tile_pool(name="ht0", bufs=1)      # f/br; reused as blend out
    trans = tc.alloc_tile_pool(name="trans", bufs=1)      # released end of phase E
    ht1pool = tc.alloc_tile_pool(name="ht1", bufs=1)      # f/b/br; released end of E
    xtpool = tc.alloc_tile_pool(name="xtilde", bufs=1)    # x~ slots shared by L0/L1
    wh1pool = tc.alloc_tile_pool(name="wh1", bufs=1)      # released end of D
    ht0tmp = tc.alloc_tile_pool(name="ht0tmp", bufs=1)    # b/fr; released end of C

    ident = const.tile([128, 128], BF16)
    make_identity(nc, ident)
    # scaled identity used to inject x~ into the fp8-scaled PSUM groups
    ident128 = const.tile([128, 128], BF16)
    make_identity(nc, ident128)
    nc.vector.tensor_scalar(out=ident128, in0=ident128, scalar1=W8SCALE,
                            scalar2=None, op0=ALU.mult)
    c0sb = const.tile([128, 4], F32)
    nc.sync.dma_start(out=c0sb, in_=dins["c0f"])
    # Engine APs must start at a 32-aligned partition, so "ones" rows living at
    # odd partitions are written via SBUF->SBUF DMA from this partition-0 tile.
    ones_c = const.tile([1, BL, T + 1], BF16)
    nc.vector.memset(ones_c, 1.0)

    def init_ht(ht):
        # fresh-SBUF init in ONE DMA (DMA instructions only support one wait):
        # zeros + h0 at slot 0 + the ones rail for the bias rows.
        nc.sync.dma_start(out=ht, in_=dins["hti"])

    # All recurrence state tensors are allocated and initialized up front, on
    # fresh SBUF, so their init DMAs carry at most one sync wait each (the DMA
    # lowering only supports a single wait condition).
    ht0 = {}
    ht1 = {}
    ht8 = {}
    ht0["f"] = ht0pool.tile([128, 4, BL, T + 1], BF16, name="ht0f", tag="ht0f")
    ht0["br"] = ht0pool.tile([128, 4, BL, T + 1], BF16, name="ht0br", tag="ht0br")
    for s in ("0f", "0b", "1f", "1b"):
        ht8[s] = ht0pool.tile([128, 4, BL, T + 1], FP8, name="ht8" + s, tag="ht8" + s)
    ht0["b"] = ht0tmp.tile([128, 4, BL, T + 1], BF16, name="ht0b", tag="ht0b")
    ht0["fr"] = ht0tmp.tile([128, 4, BL, T + 1], BF16, name="ht0fr", tag="ht0fr")
    ht1["f"] = ht1pool.tile([128, 4, BL, T + 1], BF16, name="ht1f", tag="ht1f")
    ht1["b"] = ht1pool.tile([128, 4, BL, T + 1], BF16, name="ht1b", tag="ht1b")
    ht1["br"] = ht1pool.tile([128, 4, BL, T + 1], BF16, name="ht1br", tag="ht1br")

    # -------- phase A: layer-0 x_tilde (feature-major) --------
    whpool = tc.alloc_tile_pool(name="wh0", bufs=1)
    xpool = tc.alloc_tile_pool(name="xt", bufs=1)
    xt_sb = _load_w(nc, xpool, dins["xT"], NKD, L, "xt")
    # time-reversed view of the same tile for the backward stream
    xt_rev = xt_sb.rearrange("p k (b t) -> p k b t", b=BL)[:, :, :, ::-1]
    # f-dir weights next: the first matmul can start as soon as xT + the first
    # wx0f chunk have landed
    wx0f = _load_w(nc, xpool, dins["wx0f"], NKD, 3 * GP, "wx0", nsplit=3)

    # recurrence-state init + phase-B weights, issued behind the phase-A loads
    wh0_8 = {"f": whpool.tile([128, 2, 2, 3 * GP], FP8, name="wh0f8", tag="wh0f8"),
             "b": whpool.tile([128, 2, 2, 3 * GP], FP8, name="wh0b8", tag="wh0b8")}
    nc.sync.dma_start(out=wh0_8["f"], in_=dins["wh0f8"])
    for t_ in (ht0["f"], ht0["b"], ht1["f"], ht1["b"]):
        init_ht(t_)
    for s in ("0f", "0b", "1f", "1b"):
        nc.sync.dma_start(out=ht8[s], in_=dins["hti8"])
    nc.sync.dma_start(out=wh0_8["b"], in_=dins["wh0b8"])
    wh0 = {"f": _load_w(nc, whpool, dins["wh0f"], NKH, 3 * GP, "wh0f"),
           "b": _load_w(nc, whpool, dins["wh0b"], NKH, 3 * GP, "wh0b")}
    s1T = {}
    for nm in ("s", "e"):
        st = sepool.tile([128, 2, L], BF16, name=nm + "1T", tag=nm + "1T")
        nc.sync.dma_start(out=st[F - 128:F - 127, 1, :],
                          in_=ones_c.rearrange("p b t -> p (b t)")[:, 0:L])
        s1T[nm] = st
    # highway weights (persistent; issued early so the DMA queue drains them
    # during the long Jacobi windows)
    whw = {"f": _load_w(nc, endw, dins["whwf"], NKH, 2 * GP, "whwf"),
           "b": _load_w(nc, endw, dins["whwb"], NKH, 2 * GP, "whwb")}

    def psum_tile():
        return ppool.tile([128, 4, GP], F32, name="pz", tag="pz")

    xt0 = {}
    for s in ("f", "b"):
        # both directions share one weight buffer (tag wx0): the b-dir load
        # starts as soon as the f-dir matmuls finish reading it.  The b load
        # is issued on the gpsimd DMA queue so its WAR wait doesn't
        # head-of-line-block the main (sync) DMA queue.
        wt = wx0f if s == "f" else _load_w(nc, xpool, dins["wx0b"], NKD, 3 * GP,
                                           "wx0", nsplit=3, eng=nc.gpsimd)
        store = xtpool.tile([128, NM, GP], BF16, name="xt0" + s, tag="xt" + s)
        for grp in range(3):
            pz = psum_tile()
            for mi in range(4):
                m = grp * 4 + mi
                for k in range(NKD):
                    mov = xt_sb[:, k, :] if s == "f" else xt_rev[:, k, :, :]
                    nc.tensor.matmul(pz[:, mi, :], wt[:, k, m * 128:(m + 1) * 128],
                                     mov, start=(k == 0), stop=(k == NKD - 1))
            nc.scalar.copy(store[:, grp * 4:(grp + 1) * 4, :], pz)
        xt0[s] = store
    xpool.release()

    def lstm_jacobi_pair(streams):
        """Iterate both directions' LSTM fixed points together so the two
        streams' matmuls, activations and scans overlap across engines.
        streams = [(wh_tile_bf16, wh_pairs_fp8, xs, ht_bf16, ht_fp8), ...];
        ht is [128, 4, 2, 257], pre-initialized (slot 0 = h0, ones rail at
        [16, 3], zeros).

        Iterations 0..K-2 run the recurrence matmuls in fp8 DoubleRow mode
        (2 K-tiles per instruction at 0.5 cycles/row) against the fp8 h-state;
        weights are pre-scaled by W8SCALE and the activation divides it back
        out.  The x~ injection stays bf16 (its accuracy persists into the
        fixed point).  The last iteration runs fully in bf16: the fp8
        quantization noise of earlier iterates contracts by ~4x per iteration,
        so only bf16-level noise survives in the final h."""
        for it in range(K_ITERS):
            fp8 = it < K_ITERS - 1
            for si, (wh_t, wh_p8, xs, ht, h8) in enumerate(streams):
                ss = str(si)
                mov = h8 if fp8 else ht
                # the last fp8 iteration feeds the bf16 one: write bf16 h
                wout = ht if it >= K_ITERS - 2 else h8
                I = trans.tile([128, 4, BL, T], BF16, name="I" + ss, tag="I" + ss)
                Gt = trans.tile([128, 4, BL, T], BF16, name="Gt" + ss, tag="Gt" + ss)
                O = trans.tile([128, 4, BL, T], BF16, name="O" + ss, tag="O" + ss)
                for g, (dst, fn) in enumerate(((I, AF.Sigmoid), (Gt, AF.Tanh),
                                               (O, AF.Sigmoid))):
                    pz = psum_tile()
                    for mi in range(4):
                        m = g * 4 + mi
                        nc.tensor.matmul(pz[:, mi, :], ident128 if fp8 else ident,
                                         xs[:, m, :], start=True, stop=False)
                        if fp8:
                            for pair in range(2):
                                nc.tensor.matmul(
                                    pz[:, mi, :],
                                    wh_p8[:, pair, :, m * 128:(m + 1) * 128],
                                    mov[:, 2 * pair:2 * pair + 2, :, 0:T],
                                    start=False, stop=(pair == 1), perf_mode=DR)
                        else:
                            for k in range(NKH):
                                nc.tensor.matmul(pz[:, mi, :],
                                                 wh_t[:, k, m * 128:(m + 1) * 128],
                                                 mov[:, k, :, 0:T],
                                                 start=False, stop=(k == NKH - 1))
                    nc.scalar.activation(dst, pz.rearrange("p m (b t) -> p m b t", b=BL),
                                         fn, scale=(1.0 / W8SCALE) if fp8 else 1.0)
                nc.vector.tensor_mul(Gt, I, Gt)                     # b_t = i * g
                nc.vector.tensor_scalar(out=I, in0=I, scalar1=-1.0, scalar2=1.0,
                                        op0=ALU.mult, op1=ALU.add)
                for k in range(4):                                   # c scan per (chunk, b)
                    for b in range(BL):
                        # in-place over Gt: the scan reads data1[t] before
                        # writing out[t], so out may alias data1
                        nc.vector.tensor_tensor_scan(
                            out=Gt[:, k, b, :], data0=I[:, k, b, :], data1=Gt[:, k, b, :],
                            initial=c0sb[:, k:k + 1],
                            op0=ALU.mult, op1=ALU.add)
                nc.scalar.activation(Gt, Gt, AF.Tanh)
                nc.vector.tensor_mul(wout[:, 0:3, :, 1:T + 1], Gt[:, 0:3], O[:, 0:3])
                nc.vector.tensor_mul(wout[0:16, 3, :, 1:T + 1], Gt[0:16, 3], O[0:16, 3])

    # -------- phase B: layer-0 recurrences (both directions interleaved) -----
    lstm_jacobi_pair([(wh0["f"], wh0_8["f"], xt0["f"], ht0["f"], ht8["0f"]),
                      (wh0["b"], wh0_8["b"], xt0["b"], ht0["b"], ht8["0b"])])
    whpool.release()

    # reversed-time copies (the ones rail at [16, 3] copies over too)
    nc.vector.tensor_copy(ht0["fr"][:, :, :, 1:T + 1], ht0["f"][:, :, :, T:0:-1])
    nc.vector.tensor_copy(ht0["br"][:, :, :, 1:T + 1], ht0["b"][:, :, :, T:0:-1])

    # -------- phase C: layer-1 x_tilde --------
    wx1pool = tc.alloc_tile_pool(name="wx1", bufs=1)
    wh1_8 = {"f": wh1pool.tile([128, 2, 2, 3 * GP], FP8, name="wh1f8", tag="wh1f8"),
             "b": wh1pool.tile([128, 2, 2, 3 * GP], FP8, name="wh1b8", tag="wh1b8")}
    nc.sync.dma_start(out=wh1_8["f"], in_=dins["wh1f8"])
    nc.sync.dma_start(out=wh1_8["b"], in_=dins["wh1b8"])
    wx1 = {}
    for s in ("f", "b"):
        wx1[s] = (_load_w(nc, wx1pool, dins["wx1" + s + "f"], NKH, 3 * GP, "wx1" + s + "f"),
                  _load_w(nc, wx1pool, dins["wx1" + s + "b"], NKH, 3 * GP, "wx1" + s + "b"))
    wh1 = {"f": _load_w(nc, wh1pool, dins["wh1f"], NKH, 3 * GP, "wh1f"),
           "b": _load_w(nc, wh1pool, dins["wh1b"], NKH, 3 * GP, "wh1b")}

    xt1 = {}
    for s, (hfmov, hbmov) in (("f", (ht0["f"], ht0["br"])), ("b", (ht0["fr"], ht0["b"]))):
        wtf, wtb = wx1[s]
        store = xtpool.tile([128, NM, GP], BF16, name="xt1" + s, tag="xt" + s)
        pairs = [(wtf, hfmov, k) for k in range(NKH)] + [(wtb, hbmov, k) for k in range(NKH)]
        for grp in range(3):
            pz = psum_tile()
            for mi in range(4):
                m = grp * 4 + mi
                for pi, (wt, mov, k) in enumerate(pairs):
                    nc.tensor.matmul(pz[:, mi, :], wt[:, k, m * 128:(m + 1) * 128],
                                     mov[:, k, :, 1:T + 1],
                                     start=(pi == 0), stop=(pi == 7))
            nc.scalar.copy(store[:, grp * 4:(grp + 1) * 4, :], pz)
        xt1[s] = store
    wx1pool.release()
    ht0tmp.release()

    # -------- phase D: layer-1 recurrences (both directions interleaved) -----
    lstm_jacobi_pair([(wh1["f"], wh1_8["f"], xt1["f"], ht1["f"], ht8["1f"]),
                      (wh1["b"], wh1_8["b"], xt1["b"], ht1["b"], ht8["1b"])])
    wh1pool.release()
    nc.vector.tensor_copy(ht1["br"][:, :, :, 1:T + 1], ht1["b"][:, :, :, T:0:-1])
    xtpool.release()

    # -------- phase E: highway gate + blend (in place over ht0 f/br slots) ----
    outT = {}
    pairs = [(whw["f"], ht1["f"], k) for k in range(NKH)] + \
            [(whw["b"], ht1["br"], k) for k in range(NKH)]
    for half, (h1, h0) in (("f", (ht1["f"], ht0["f"])), ("b", (ht1["br"], ht0["br"]))):
        pz = psum_tile()
        for mi in range(4):
            m = (0 if half == "f" else 4) + mi
            for pi, (wt, mov, k) in enumerate(pairs):
                nc.tensor.matmul(pz[:, mi, :], wt[:, k, m * 128:(m + 1) * 128],
                                 mov[:, k, :, 1:T + 1],
                                 start=(pi == 0), stop=(pi == 7))
        gate = trans.tile([128, 4, BL, T], BF16, name="gate", tag="I0")
        nc.scalar.activation(gate, pz.rearrange("p m (b t) -> p m b t", b=BL), AF.Sigmoid)
        tmp = trans.tile([128, 4, BL, T], BF16, name="tmpb", tag="Gt0")
        hsl = h0[:, :, :, 1:T + 1]
        nc.vector.tensor_sub(tmp, h1[:, :, :, 1:T + 1], hsl)
        nc.vector.tensor_mul(tmp, gate, tmp)
        # the final write skips partition 16 of chunk 3 so the ones rail from
        # the init image survives for the projection bias rows
        nc.vector.tensor_add(hsl[:, 0:3], hsl[:, 0:3], tmp[:, 0:3])
        nc.vector.tensor_add(hsl[0:16, 3], hsl[0:16, 3], tmp[0:16, 3])
        outT[half] = h0
    ht1pool.release()
    trans.release()

    # -------- phase F: s/e projections --------
    latew = tc.alloc_tile_pool(name="latew", bufs=1)
    wse = {}
    for nm in ("s", "e"):
        wse[nm] = {"f": _load_w(nc, latew, dins["w" + nm + "f"], NKH, F, "w" + nm + "f"),
                   "b": _load_w(nc, latew, dins["w" + nm + "b"], NKH, F, "w" + nm + "b")}
    ut = _load_w(nc, latew, dins["upk"], 2, C * 256, "upk")
    for nm in ("s", "e"):
        wf, wb = wse[nm]["f"], wse[nm]["b"]
        st = s1T[nm]
        prs = [(wf, outT["f"], k) for k in range(NKH)] + [(wb, outT["b"], k) for k in range(NKH)]
        pz = psum_tile()
        for mi, (ma, mb) in enumerate(((0, 128), (128, F))):
            for pi, (wt, mov, k) in enumerate(prs):
                nc.tensor.matmul(pz[0:mb - ma, mi, :], wt[:, k, ma:mb],
                                 mov[:, k, :, 1:T + 1],
                                 start=(pi == 0), stop=(pi == 7))
        nc.scalar.copy(st[:, 0, :], pz[:, 0, :])
        nc.scalar.copy(st[0:F - 128, 1, :], pz[0:F - 128, 1, :])

    # -------- phase G: biaffine part 1: tmp[(c,j), (b,t)] --------
    biapool = tc.alloc_tile_pool(name="bia", bufs=1)
    smov = [s1T["s"][:, 0, :], s1T["s"][0:F + 1 - 128, 1, :]]
    ut_t = [ut[:, 0, :], ut[0:F + 1 - 128, 1, :]]
    tmpT = biapool.tile([128, 16, GP], BF16, name="tmpT", tag="tmpT")
    for grp in range(4):
        pz = psum_tile()
        for mi in range(4):
            m = grp * 4 + mi
            for k in range(2):
                nc.tensor.matmul(pz[:, mi, :], ut_t[k][:, m * 128:(m + 1) * 128],
                                 smov[k], start=(k == 0), stop=(k == 1))
        nc.scalar.copy(tmpT[:, grp * 4:(grp + 1) * 4, :], pz)

    # -------- phase H: biaffine part 2 + output assembly --------
    emov0 = s1T["e"][:, 0, :].rearrange("p (b t) -> p b t", b=BL)
    emov1 = s1T["e"][0:F + 1 - 128, 1, :].rearrange("p (b t) -> p b t", b=BL)
    ssbpool = tc.alloc_tile_pool(name="osb", bufs=2)
    for bi in range(BL):
        for xt_i in range(2):
            osb = ssbpool.tile([128, T, C], BF16, name="osb", tag="osb")
            pz = psum_tile()
            for c in range(C):
                xsl = slice(bi * T + xt_i * 128, bi * T + xt_i * 128 + 128)
                po = pz[:, c // 2, (c % 2) * T:(c % 2) * T + T]
                nc.tensor.matmul(po, tmpT[:, 2 * c, xsl], emov0[:, bi, :],
                                 start=True, stop=False)
                nc.tensor.matmul(po, tmpT[0:F + 1 - 128, 2 * c + 1, xsl],
                                 emov1[:, bi, :], start=False, stop=True)
            # one merged copy per block: psum [128, 4, 2, 256] -> osb [t, c]
            eng = nc.vector if (bi * 2 + xt_i) % 2 == 0 else nc.scalar
            if eng is nc.vector:
                nc.vector.tensor_copy(
                    osb.rearrange("p t (chi clo) -> p chi clo t", clo=2),
                    pz.rearrange("p m (clo t) -> p m clo t", clo=2))
            else:
                nc.scalar.copy(
                    osb.rearrange("p t (chi clo) -> p chi clo t", clo=2),
                    pz.rearrange("p m (clo t) -> p m clo t", clo=2))
            nc.sync.dma_start(out=out_d[bi, xt_i * 128:(xt_i + 1) * 128, :, :], in_=osb)
    ssbpool.release()
    biapool.release()
    latew.release()
    ht0pool.release()
    sepool.release()
    endw.release()
    ppool.release()
    const.release()


# ------------------------------------------------------------------ entry point

TRACE = False          # set True (from test harnesses) to capture an NTFF profile
LAST_RESULT = None     # BassKernelResults of the most recent run


def kernel(**inputs) -> np.ndarray:
    global LAST_RESULT
    if "nc" not in _CACHE:
        _CACHE["nc"] = _build_program()
    nc = _CACHE["nc"]
    in_maps = _pack_inputs(inputs)
    try:
        res = run_bass_kernel_spmd(nc, in_maps, core_ids=list(range(NCORES)),
                                   trace=TRACE)
    except ModuleNotFoundError:
        # no NTFF profile hook in this container; run without tracing
        res = run_bass_kernel_spmd(nc, in_maps, core_ids=list(range(NCORES)))
    LAST_RESULT = res
    out = np.concatenate([np.asarray(res.results[c]["out"]) for c in range(NCORES)],
                         axis=0)
    return np.ascontiguousarray(out.astype(np.float32))


if __name__ == "__main__":
    raise SystemExit("use test.py")


# revision 37
# speedup vs baseline: 1.0602x; 1.0052x over previous
"""Biaffine NER model (2-layer BiLSTM + highway + biaffine) on 8 Trainium2 cores.

Strategy:
  - Data-parallel over batch: each of the 8 cores handles B_loc=2 of the 16
    batch elements, full model, no collectives.
  - The LSTM recurrences are solved by fixed-point (Jacobi) iteration:
      H^{k+1} = LSTMCell(x_tilde + shift(H^k) @ W_h)
    Each iteration is fully parallel over time (big matmuls, M = B_loc*T = 512
    rows), and the cell-state recurrence c_t = a_t*c_{t-1} + b_t is computed
    with the hardware tensor_tensor_scan. The map contracts by ~4x per
    iteration; K_ITERS=5 sits at ~9.5e-3 rel absmax vs the 2e-2 gate.
  - Everything on-chip is kept "transposed" (feature-major, [128-partition
    folds, (b, t) free]) so matmuls, activations and scans all operate on
    full-width tiles.
  - All contraction dims are zero-padded to multiples of 128 host-side so
    every matmul uses uniform full-128 K-tiles (padding rows are zero on both
    the stationary and moving side).
  - Elementwise gate math runs in bf16 (DVE 2x/4x perf modes).
  - Biases ride as an extra contraction row (ones rail in the moving operand,
    bias row in the stationary operand).
  - Output is DMA'd as bf16 and upcast host-side (halves the output-write
    tail; adds <4e-4 rel err).
"""

import sys

sys.path.insert(0, "/opt/trn_rl_repo")

import ml_dtypes
import numpy as np

import concourse.bass as bass
import concourse.mybir as mybir
import concourse.tile as tile
from concourse.bass_utils import run_bass_kernel_spmd
from concourse.masks import make_identity

F32 = mybir.dt.float32
BF16 = mybir.dt.bfloat16
FP8 = mybir.dt.float8e4
BF16NP = ml_dtypes.bfloat16
F8NP = ml_dtypes.float8_e4m3
AF = mybir.ActivationFunctionType
ALU = mybir.AluOpType
DR = mybir.MatmulPerfMode.DoubleRow
W8SCALE = 128.0           # fp8 weight pre-scale (e4m3 max-normal is 240)

B, T, D = 16, 256, 768
H, H2, G = 400, 800, 1200
F, C = 150, 8
NCORES = 8
BL = B // NCORES          # 2 batch elements per core
L = BL * T                # 512 (b, t) rows per core
GP = 512                  # per-gate padded stride (3*GP = 1536, 12 M-tiles)
NM = 12                   # M-tiles of the padded gate dim
NKH = 4                   # K-tiles of the padded [H+1->512] contraction
NKD = 6                   # K-tiles of D=768
K_ITERS = 5

_CACHE = {}


# ------------------------------------------------------------------ host packing

def _pack_gate_cols(w):
    """[K, 3H] -> [K, 3*GP] with each gate's 400 cols padded to 512."""
    k = w.shape[0]
    out = np.zeros((k, 3 * GP), np.float32)
    for g in range(3):
        out[:, g * GP:g * GP + H] = w[:, g * H:(g + 1) * H]
    return out


def _with_bias_row(w, bias):
    """Append one row (the bias, packed like w's columns) to w."""
    return np.concatenate([w, bias[None, :]], 0)


def _fold_k(w, nk):
    """[K<=128*nk, C] -> [128, nk, C] zero-padded row fold (row r -> [r%128, r//128])."""
    k, c = w.shape
    out = np.zeros((128 * nk, c), np.float32)
    out[:k] = w
    return np.ascontiguousarray(out.reshape(nk, 128, c).transpose(1, 0, 2))


def _fold128(v, nchunk):
    """[n] -> [128, nchunk] column-major fold (unit u -> [u%128, u//128])."""
    out = np.zeros((128, nchunk), np.float32)
    n = len(v)
    for m in range(nchunk):
        seg = v[m * 128:min((m + 1) * 128, n)]
        out[:len(seg), m] = seg
    return out


def _pack_inputs(inputs):
    """Pack weights into the DRAM layouts the program expects (shared by all cores)."""
    f32 = lambda a: np.ascontiguousarray(np.asarray(a, np.float32))
    x = f32(inputs["x"])
    z = np.zeros((3 * GP,), np.float32)

    packs = {}
    fp8packs = {}

    def _fp8_pairs(whfold):
        """[128, 4, C] bf-side fold -> [128, 2(pair), 2(slot), C] fp8, x128."""
        w8 = np.clip(whfold * W8SCALE, -240.0, 240.0).astype(F8NP)
        return np.ascontiguousarray(w8.reshape(128, 2, 2, -1))

    # layer 0: W [D+H, 3H].  The gate bias rides as the last row of wh (it is
    # re-added every Jacobi iteration through the ones slot of ht).
    for nm, wn, bn in (("0f", "W_f0", "b_f0"), ("0b", "W_b0", "b_b0")):
        W = f32(inputs[wn]); bias = _pack_gate_cols(f32(inputs[bn])[None, :])[0]
        packs["wx" + nm] = _fold_k(_pack_gate_cols(W[:D]), NKD)
        wh = _fold_k(_with_bias_row(_pack_gate_cols(W[D:]), bias), NKH)
        packs["wh" + nm] = wh
        fp8packs["wh" + nm + "8"] = _fp8_pairs(wh)
    # layer 1: W [2H+H, 3H]; the input half splits into hf/hb parts (both with
    # zero bias rows -- the bias lives only in wh).
    for nm, wn, bn in (("1f", "W_f1", "b_f1"), ("1b", "W_b1", "b_b1")):
        W = f32(inputs[wn]); bias = _pack_gate_cols(f32(inputs[bn])[None, :])[0]
        packs["wx" + nm + "f"] = _fold_k(_with_bias_row(_pack_gate_cols(W[:H]), z), NKH)
        packs["wx" + nm + "b"] = _fold_k(_with_bias_row(_pack_gate_cols(W[H:H2]), z), NKH)
        wh = _fold_k(_with_bias_row(_pack_gate_cols(W[H2:]), bias), NKH)
        packs["wh" + nm] = wh
        fp8packs["wh" + nm + "8"] = _fp8_pairs(wh)

    # highway: W_hw [2H, 2H]; M packed as [f-half pad 512 | b-half pad 512]
    Whw = f32(inputs["W_hw"]); bhw = f32(inputs["b_hw"])

    def _pack_hw_cols(w):
        k = w.shape[0]
        out = np.zeros((k, 2 * GP), np.float32)
        out[:, 0:H] = w[:, 0:H]
        out[:, GP:GP + H] = w[:, H:H2]
        return out

    zh = np.zeros((2 * GP,), np.float32)
    packs["whwf"] = _fold_k(_with_bias_row(_pack_hw_cols(Whw[:H]), _pack_hw_cols(bhw[None, :])[0]), NKH)
    packs["whwb"] = _fold_k(_with_bias_row(_pack_hw_cols(Whw[H:]), zh), NKH)

    # projections: Ws/We [2H, F]
    for nm, wn, bn in (("s", "W_s", "b_s"), ("e", "W_e", "b_e")):
        W = f32(inputs[wn]); bias = f32(inputs[bn])
        packs["w" + nm + "f"] = _fold_k(_with_bias_row(W[:H], bias), NKH)
        packs["w" + nm + "b"] = _fold_k(_with_bias_row(W[H:], np.zeros((F,), np.float32)), NKH)

    # biaffine U [F+1, C, F+1] -> [F+1, C*256] (each c padded 151->256)
    U = f32(inputs["U"])
    upk = np.zeros((F + 1, C * 256), np.float32)
    for c in range(C):
        upk[:, c * 256:c * 256 + F + 1] = U[:, c, :]
    packs["upk"] = _fold_k(upk, 2)

    packs = {k: v.astype(BF16NP) for k, v in packs.items()}
    packs.update(fp8packs)
    h0f = _fold128(f32(inputs["h0"])[0], 4)
    hti = np.zeros((128, 4, BL, T + 1), np.float32)
    hti[:, :, :, 0] = h0f[:, :, None]          # slot 0 = h0
    hti[16, 3, :, :] = 1.0                     # ones rail for the bias rows
    packs["hti"] = hti.astype(BF16NP)
    packs["hti8"] = hti.astype(F8NP)
    packs["c0f"] = _fold128(f32(inputs["c0"])[0], 4)

    # per-core x, feature-major [128, 6, L]
    per_core = []
    for c in range(NCORES):
        sl = x[c * BL:(c + 1) * BL]
        m = dict(packs)
        m["xT"] = _fold_k(sl.transpose(2, 0, 1).reshape(D, L), NKD).astype(BF16NP)
        per_core.append(m)
    return per_core


# ------------------------------------------------------------------ program

def _build_program():
    nc = bass.Bass(trn_type="TRN2", target_bir_lowering=False, debug=False)

    dins = {}

    def din(name, shape, dt=BF16):
        dins[name] = nc.dram_tensor(name, list(shape), dt, kind="ExternalInput").ap()
        return dins[name]

    din("xT", (128, NKD, L))
    din("wx0f", (128, NKD, 3 * GP)); din("wx0b", (128, NKD, 3 * GP))
    din("wh0f", (128, NKH, 3 * GP)); din("wh0b", (128, NKH, 3 * GP))
    for s in ("1f", "1b"):
        din("wx" + s + "f", (128, NKH, 3 * GP))
        din("wx" + s + "b", (128, NKH, 3 * GP))
        din("wh" + s, (128, NKH, 3 * GP))
    for s in ("0f", "0b", "1f", "1b"):
        din("wh" + s + "8", (128, 2, 2, 3 * GP), dt=FP8)
    din("whwf", (128, NKH, 2 * GP)); din("whwb", (128, NKH, 2 * GP))
    din("wsf", (128, NKH, F)); din("wsb", (128, NKH, F))
    din("wef", (128, NKH, F)); din("web", (128, NKH, F))
    din("upk", (128, 2, C * 256))
    din("hti", (128, 4, BL, T + 1)); din("hti8", (128, 4, BL, T + 1), dt=FP8)
    din("c0f", (128, 4), dt=F32)
    out_d = nc.dram_tensor("out", [BL, T, T, C], BF16, kind="ExternalOutput").ap()

    with tile.TileContext(nc) as tc:
        _body(nc, tc, dins, out_d)
    _split_multi_waits(nc)
    return nc


def _split_multi_waits(nc, max_waits=1):
    """This container's walrus supports only one embedded sync-wait per
    instruction ("Too many sync wait commands"); hoist extra waits onto
    single-wait NoOps inserted just before, on the same engine queue.
    Sequential waiting on monotone semaphores is equivalent to the joint
    wait."""
    n = 0
    for func in nc.m.functions:
        for blk in func.blocks:
            out = []
            for inst in blk.instructions:
                si = inst.sync_info
                if si is not None and si.on_wait and len(si.on_wait) > max_waits:
                    waits = list(si.on_wait)
                    for j, w in enumerate(waits[:-max_waits]):
                        nop = mybir.InstNoOp(name=f"{inst.name}-xw{j}")
                        nop.engine = inst.engine
                        nop.sync_info = mybir.SyncInfo(on_wait=[w], on_update=[])
                        out.append(nop)
                        n += 1
                    inst.sync_info = mybir.SyncInfo(
                        on_wait=waits[-max_waits:], on_update=list(si.on_update))
                out.append(inst)
            blk.instructions = out
    return n


def _load_w(nc, pool, dram, nk, cols, tag, nsplit=1, eng=None):
    """One [128, nk, cols] tile; loaded via `nsplit` DMAs along the k axis."""
    t = pool.tile([128, nk, cols], BF16, name=tag, tag=tag)
    step = (nk + nsplit - 1) // nsplit
    for a in range(0, nk, step):
        b = min(a + step, nk)
        (eng or nc.sync).dma_start(out=t[:, a:b, :], in_=dram[:, a:b, :])
    return t


def _body(nc, tc, dins, out_d):
    # Pool allocation order is the (LIFO) release order, reversed.  Base pools
    # live to the end; big transients nest inside phase windows.
    const = tc.alloc_tile_pool(name="const", bufs=1)
    ppool = tc.alloc_tile_pool(name="psum", bufs=2, space="PSUM")
    endw = tc.alloc_tile_pool(name="endw", bufs=1)        # endgame weights
    sepool = tc.alloc_tile_pool(name="se", bufs=1)        # s1/e1 (+ early ones rows)
    ht0pool = tc.alloc_tile_pool(name="ht0", bufs=1)      # f/br; reused as blend out
    trans = tc.alloc_tile_pool(name="trans", bufs=1)      # released end of phase E
    ht1pool = tc.alloc_tile_pool(name="ht1", bufs=1)      # f/b/br; released end of E
    xtpool = tc.alloc_tile_pool(name="xtilde", bufs=1)    # x~ slots shared by L0/L1
    wh1pool = tc.alloc_tile_pool(name="wh1", bufs=1)      # released end of D
    ht0tmp = tc.alloc_tile_pool(name="ht0tmp", bufs=1)    # b/fr; released end of C

    ident = const.tile([128, 128], BF16)
    make_identity(nc, ident)
    # scaled identity used to inject x~ into the fp8-scaled PSUM groups
    ident128 = const.tile([128, 128], BF16)
    make_identity(nc, ident128)
    nc.vector.tensor_scalar(out=ident128, in0=ident128, scalar1=W8SCALE,
                            scalar2=None, op0=ALU.mult)
    c0sb = const.tile([128, 4], F32)
    nc.sync.dma_start(out=c0sb, in_=dins["c0f"])
    # Engine APs must start at a 32-aligned partition, so "ones" rows living at
    # odd partitions are written via SBUF->SBUF DMA from this partition-0 tile.
    ones_c = const.tile([1, BL, T + 1], BF16)
    nc.vector.memset(ones_c, 1.0)

    def init_ht(ht):
        # fresh-SBUF init in ONE DMA (DMA instructions only support one wait):
        # zeros + h0 at slot 0 + the ones rail for the bias rows.
        nc.sync.dma_start(out=ht, in_=dins["hti"])

    # All recurrence state tensors are allocated and initialized up front, on
    # fresh SBUF, so their init DMAs carry at most one sync wait each (the DMA
    # lowering only supports a single wait condition).
    ht0 = {}
    ht1 = {}
    ht8 = {}
    ht0["f"] = ht0pool.tile([128, 4, BL, T + 1], BF16, name="ht0f", tag="ht0f")
    ht0["br"] = ht0pool.tile([128, 4, BL, T + 1], BF16, name="ht0br", tag="ht0br")
    for s in ("0f", "0b", "1f", "1b"):
        ht8[s] = ht0pool.tile([128, 4, BL, T + 1], FP8, name="ht8" + s, tag="ht8" + s)
    ht0["b"] = ht0tmp.tile([128, 4, BL, T + 1], BF16, name="ht0b", tag="ht0b")
    ht0["fr"] = ht0tmp.tile([128, 4, BL, T + 1], BF16, name="ht0fr", tag="ht0fr")
    ht1["f"] = ht1pool.tile([128, 4, BL, T + 1], BF16, name="ht1f", tag="ht1f")
    ht1["b"] = ht1pool.tile([128, 4, BL, T + 1], BF16, name="ht1b", tag="ht1b")
    ht1["br"] = ht1pool.tile([128, 4, BL, T + 1], BF16, name="ht1br", tag="ht1br")
    wh1 = {}
    wh1_8 = {"f": wh1pool.tile([128, 2, 2, 3 * GP], FP8, name="wh1f8", tag="wh1f8"),
             "b": wh1pool.tile([128, 2, 2, 3 * GP], FP8, name="wh1b8", tag="wh1b8")}

    # -------- phase A: layer-0 x_tilde (feature-major) --------
    whpool = tc.alloc_tile_pool(name="wh0", bufs=1)
    xpool = tc.alloc_tile_pool(name="xt", bufs=1)
    xt_sb = _load_w(nc, xpool, dins["xT"], NKD, L, "xt")
    # time-reversed view of the same tile for the backward stream
    xt_rev = xt_sb.rearrange("p k (b t) -> p k b t", b=BL)[:, :, :, ::-1]
    # f-dir weights next: the first matmul can start as soon as xT + the first
    # wx0f chunk have landed
    wx0f = _load_w(nc, xpool, dins["wx0f"], NKD, 3 * GP, "wx0", nsplit=3)

    # recurrence-state init + phase-B fp8 weights, issued behind the phase-A
    # loads.  Everything needed later (bf16 wh for the final iterations, the
    # layer-1 state inits, the highway weights) is issued from inside the
    # Jacobi windows so it never delays the phase-A/B critical path.
    wh0_8 = {"f": whpool.tile([128, 2, 2, 3 * GP], FP8, name="wh0f8", tag="wh0f8"),
             "b": whpool.tile([128, 2, 2, 3 * GP], FP8, name="wh0b8", tag="wh0b8")}
    nc.sync.dma_start(out=wh0_8["f"], in_=dins["wh0f8"])
    for t_ in (ht0["f"], ht0["b"]):
        init_ht(t_)
    for s in ("0f", "0b"):
        nc.sync.dma_start(out=ht8[s], in_=dins["hti8"])
    nc.sync.dma_start(out=wh0_8["b"], in_=dins["wh0b8"])
    wh0 = {}
    s1T = {}
    for nm in ("s", "e"):
        st = sepool.tile([128, 2, L], BF16, name=nm + "1T", tag=nm + "1T")
        nc.sync.dma_start(out=st[F - 128:F - 127, 1, :],
                          in_=ones_c.rearrange("p b t -> p (b t)")[:, 0:L])
        s1T[nm] = st
    whw = {}

    def deferred_b_loads():
        # issued after the first Jacobi-L0 iteration is emitted: the bf16 wh0
        # is only read by the final (bf16) iteration, ~4 iterations later
        wh0["f"] = _load_w(nc, whpool, dins["wh0f"], NKH, 3 * GP, "wh0f")
        wh0["b"] = _load_w(nc, whpool, dins["wh0b"], NKH, 3 * GP, "wh0b")
        nc.sync.dma_start(out=wh1_8["f"], in_=dins["wh1f8"])
        nc.sync.dma_start(out=wh1_8["b"], in_=dins["wh1b8"])
        for t_ in (ht1["f"], ht1["b"]):
            init_ht(t_)
        for s in ("1f", "1b"):
            nc.sync.dma_start(out=ht8[s], in_=dins["hti8"])
        whw["f"] = _load_w(nc, endw, dins["whwf"], NKH, 2 * GP, "whwf")
        whw["b"] = _load_w(nc, endw, dins["whwb"], NKH, 2 * GP, "whwb")
        wh1["f"] = _load_w(nc, wh1pool, dins["wh1f"], NKH, 3 * GP, "wh1f")
        wh1["b"] = _load_w(nc, wh1pool, dins["wh1b"], NKH, 3 * GP, "wh1b")

    def psum_tile():
        return ppool.tile([128, 4, GP], F32, name="pz", tag="pz")

    xt0 = {}
    for s in ("f", "b"):
        # both directions share one weight buffer (tag wx0): the b-dir load
        # starts as soon as the f-dir matmuls finish reading it.  The b load
        # is issued on the gpsimd DMA queue so its WAR wait doesn't
        # head-of-line-block the main (sync) DMA queue.
        wt = wx0f if s == "f" else _load_w(nc, xpool, dins["wx0b"], NKD, 3 * GP,
                                           "wx0", nsplit=3, eng=nc.gpsimd)
        store = xtpool.tile([128, NM, GP], BF16, name="xt0" + s, tag="xt" + s)
        for grp in range(3):
            pz = psum_tile()
            for mi in range(4):
                m = grp * 4 + mi
                for k in range(NKD):
                    mov = xt_sb[:, k, :] if s == "f" else xt_rev[:, k, :, :]
                    nc.tensor.matmul(pz[:, mi, :], wt[:, k, m * 128:(m + 1) * 128],
                                     mov, start=(k == 0), stop=(k == NKD - 1))
            nc.scalar.copy(store[:, grp * 4:(grp + 1) * 4, :], pz)
        xt0[s] = store
    xpool.release()

    def lstm_jacobi_pair(streams, after_first_iter=None):
        """Iterate both directions' LSTM fixed points together so the two
        streams' matmuls, activations and scans overlap across engines.
        streams = [(wh_dict, key, wh_pairs_fp8, xs, ht_bf16, ht_fp8), ...];
        wh_dict[key] (bf16 weights) is resolved lazily at the final
        iteration, so its DMA may be issued via after_first_iter (called
        after iteration 0's program is emitted);
        ht is [128, 4, 2, 257], pre-initialized (slot 0 = h0, ones rail at
        [16, 3], zeros).

        Iterations 0..K-2 run the recurrence matmuls in fp8 DoubleRow mode
        (2 K-tiles per instruction at 0.5 cycles/row) against the fp8 h-state;
        weights are pre-scaled by W8SCALE and the activation divides it back
        out.  The x~ injection stays bf16 (its accuracy persists into the
        fixed point).  The last iteration runs fully in bf16: the fp8
        quantization noise of earlier iterates contracts by ~4x per iteration,
        so only bf16-level noise survives in the final h."""
        for it in range(K_ITERS):
            if it == 1 and after_first_iter is not None:
                after_first_iter()
            fp8 = it < K_ITERS - 1
            for si, (wh_d, wh_k, wh_p8, xs, ht, h8) in enumerate(streams):
                ss = str(si)
                mov = h8 if fp8 else ht
                # the last fp8 iteration feeds the bf16 one: write bf16 h
                wout = ht if it >= K_ITERS - 2 else h8
                I = trans.tile([128, 4, BL, T], BF16, name="I" + ss, tag="I" + ss)
                Gt = trans.tile([128, 4, BL, T], BF16, name="Gt" + ss, tag="Gt" + ss)
                O = trans.tile([128, 4, BL, T], BF16, name="O" + ss, tag="O" + ss)
                for g, (dst, fn) in enumerate(((I, AF.Sigmoid), (Gt, AF.Tanh),
                                               (O, AF.Sigmoid))):
                    pz = psum_tile()
                    for mi in range(4):
                        m = g * 4 + mi
                        nc.tensor.matmul(pz[:, mi, :], ident128 if fp8 else ident,
                                         xs[:, m, :], start=True, stop=False)
                        if fp8:
                            for pair in range(2):
                                nc.tensor.matmul(
                                    pz[:, mi, :],
                                    wh_p8[:, pair, :, m * 128:(m + 1) * 128],
                                    mov[:, 2 * pair:2 * pair + 2, :, 0:T],
                                    start=False, stop=(pair == 1), perf_mode=DR)
                        else:
                            for k in range(NKH):
                                nc.tensor.matmul(pz[:, mi, :],
                                                 wh_d[wh_k][:, k, m * 128:(m + 1) * 128],
                                                 mov[:, k, :, 0:T],
                                                 start=False, stop=(k == NKH - 1))
                    nc.scalar.activation(dst, pz.rearrange("p m (b t) -> p m b t", b=BL),
                                         fn, scale=(1.0 / W8SCALE) if fp8 else 1.0)
                nc.vector.tensor_mul(Gt, I, Gt)                     # b_t = i * g
                nc.vector.tensor_scalar(out=I, in0=I, scalar1=-1.0, scalar2=1.0,
                                        op0=ALU.mult, op1=ALU.add)
                for k in range(4):                                   # c scan per (chunk, b)
                    for b in range(BL):
                        # in-place over Gt: the scan reads data1[t] before
                        # writing out[t], so out may alias data1
                        nc.vector.tensor_tensor_scan(
                            out=Gt[:, k, b, :], data0=I[:, k, b, :], data1=Gt[:, k, b, :],
                            initial=c0sb[:, k:k + 1],
                            op0=ALU.mult, op1=ALU.add)
                nc.scalar.activation(Gt, Gt, AF.Tanh)
                nc.vector.tensor_mul(wout[:, 0:3, :, 1:T + 1], Gt[:, 0:3], O[:, 0:3])
                nc.vector.tensor_mul(wout[0:16, 3, :, 1:T + 1], Gt[0:16, 3], O[0:16, 3])

    # -------- phase B: layer-0 recurrences (both directions interleaved) -----
    lstm_jacobi_pair([(wh0, "f", wh0_8["f"], xt0["f"], ht0["f"], ht8["0f"]),
                      (wh0, "b", wh0_8["b"], xt0["b"], ht0["b"], ht8["0b"])],
                     after_first_iter=deferred_b_loads)
    whpool.release()

    # reversed-time copies (the ones rail at [16, 3] copies over too)
    nc.vector.tensor_copy(ht0["fr"][:, :, :, 1:T + 1], ht0["f"][:, :, :, T:0:-1])
    nc.vector.tensor_copy(ht0["br"][:, :, :, 1:T + 1], ht0["b"][:, :, :, T:0:-1])

    # -------- phase C: layer-1 x_tilde --------
    wx1pool = tc.alloc_tile_pool(name="wx1", bufs=1)
    wx1 = {}
    for s in ("f", "b"):
        wx1[s] = (_load_w(nc, wx1pool, dins["wx1" + s + "f"], NKH, 3 * GP,
                          "wx1" + s + "f", eng=nc.gpsimd),
                  _load_w(nc, wx1pool, dins["wx1" + s + "b"], NKH, 3 * GP,
                          "wx1" + s + "b", eng=nc.gpsimd))

    xt1 = {}
    for s, (hfmov, hbmov) in (("f", (ht0["f"], ht0["br"])), ("b", (ht0["fr"], ht0["b"]))):
        wtf, wtb = wx1[s]
        store = xtpool.tile([128, NM, GP], BF16, name="xt1" + s, tag="xt" + s)
        pairs = [(wtf, hfmov, k) for k in range(NKH)] + [(wtb, hbmov, k) for k in range(NKH)]
        for grp in range(3):
            pz = psum_tile()
            for mi in range(4):
                m = grp * 4 + mi
                for pi, (wt, mov, k) in enumerate(pairs):
                    nc.tensor.matmul(pz[:, mi, :], wt[:, k, m * 128:(m + 1) * 128],
                                     mov[:, k, :, 1:T + 1],
                                     start=(pi == 0), stop=(pi == 7))
            nc.scalar.copy(store[:, grp * 4:(grp + 1) * 4, :], pz)
        xt1[s] = store
    wx1pool.release()
    ht0tmp.release()

    # -------- phase D: layer-1 recurrences (both directions interleaved) -----
    lstm_jacobi_pair([(wh1, "f", wh1_8["f"], xt1["f"], ht1["f"], ht8["1f"]),
                      (wh1, "b", wh1_8["b"], xt1["b"], ht1["b"], ht8["1b"])])
    wh1pool.release()
    nc.vector.tensor_copy(ht1["br"][:, :, :, 1:T + 1], ht1["b"][:, :, :, T:0:-1])
    xtpool.release()

    # -------- phase E: highway gate + blend (in place over ht0 f/br slots) ----
    outT = {}
    pairs = [(whw["f"], ht1["f"], k) for k in range(NKH)] + \
            [(whw["b"], ht1["br"], k) for k in range(NKH)]
    for half, (h1, h0) in (("f", (ht1["f"], ht0["f"])), ("b", (ht1["br"], ht0["br"]))):
        pz = psum_tile()
        for mi in range(4):
            m = (0 if half == "f" else 4) + mi
            for pi, (wt, mov, k) in enumerate(pairs):
                nc.tensor.matmul(pz[:, mi, :], wt[:, k, m * 128:(m + 1) * 128],
                                 mov[:, k, :, 1:T + 1],
                                 start=(pi == 0), stop=(pi == 7))
        gate = trans.tile([128, 4, BL, T], BF16, name="gate", tag="I0")
        nc.scalar.activation(gate, pz.rearrange("p m (b t) -> p m b t", b=BL), AF.Sigmoid)
        tmp = trans.tile([128, 4, BL, T], BF16, name="tmpb", tag="Gt0")
        hsl = h0[:, :, :, 1:T + 1]
        nc.vector.tensor_sub(tmp, h1[:, :, :, 1:T + 1], hsl)
        nc.vector.tensor_mul(tmp, gate, tmp)
        # the final write skips partition 16 of chunk 3 so the ones rail from
        # the init image survives for the projection bias rows
        nc.vector.tensor_add(hsl[:, 0:3], hsl[:, 0:3], tmp[:, 0:3])
        nc.vector.tensor_add(hsl[0:16, 3], hsl[0:16, 3], tmp[0:16, 3])
        outT[half] = h0
    ht1pool.release()
    trans.release()

    # -------- phase F: s/e projections --------
    latew = tc.alloc_tile_pool(name="latew", bufs=1)
    wse = {}
    for nm in ("s", "e"):
        wse[nm] = {"f": _load_w(nc, latew, dins["w" + nm + "f"], NKH, F, "w" + nm + "f"),
                   "b": _load_w(nc, latew, dins["w" + nm + "b"], NKH, F, "w" + nm + "b")}
    ut = _load_w(nc, latew, dins["upk"], 2, C * 256, "upk")
    for nm in ("s", "e"):
        wf, wb = wse[nm]["f"], wse[nm]["b"]
        st = s1T[nm]
        prs = [(wf, outT["f"], k) for k in range(NKH)] + [(wb, outT["b"], k) for k in range(NKH)]
        pz = psum_tile()
        for mi, (ma, mb) in enumerate(((0, 128), (128, F))):
            for pi, (wt, mov, k) in enumerate(prs):
                nc.tensor.matmul(pz[0:mb - ma, mi, :], wt[:, k, ma:mb],
                                 mov[:, k, :, 1:T + 1],
                                 start=(pi == 0), stop=(pi == 7))
        nc.scalar.copy(st[:, 0, :], pz[:, 0, :])
        nc.scalar.copy(st[0:F - 128, 1, :], pz[0:F - 128, 1, :])

    # -------- phase G: biaffine part 1: tmp[(c,j), (b,t)] --------
    biapool = tc.alloc_tile_pool(name="bia", bufs=1)
    smov = [s1T["s"][:, 0, :], s1T["s"][0:F + 1 - 128, 1, :]]
    ut_t = [ut[:, 0, :], ut[0:F + 1 - 128, 1, :]]
    tmpT = biapool.tile([128, 16, GP], BF16, name="tmpT", tag="tmpT")
    for grp in range(4):
        pz = psum_tile()
        for mi in range(4):
            m = grp * 4 + mi
            for k in range(2):
                nc.tensor.matmul(pz[:, mi, :], ut_t[k][:, m * 128:(m + 1) * 128],
                                 smov[k], start=(k == 0), stop=(k == 1))
        nc.scalar.copy(tmpT[:, grp * 4:(grp + 1) * 4, :], pz)

    # -------- phase H: biaffine part 2 + output assembly --------
    emov0 = s1T["e"][:, 0, :].rearrange("p (b t) -> p b t", b=BL)
    emov1 = s1T["e"][0:F + 1 - 128, 1, :].rearrange("p (b t) -> p b t", b=BL)
    ssbpool = tc.alloc_tile_pool(name="osb", bufs=2)
    for bi in range(BL):
        for xt_i in range(2):
            osb = ssbpool.tile([128, T, C], BF16, name="osb", tag="osb")
            pz = psum_tile()
            for c in range(C):
                xsl = slice(bi * T + xt_i * 128, bi * T + xt_i * 128 + 128)
                po = pz[:, c // 2, (c % 2) * T:(c % 2) * T + T]
                nc.tensor.matmul(po, tmpT[:, 2 * c, xsl], emov0[:, bi, :],
                                 start=True, stop=False)
                nc.tensor.matmul(po, tmpT[0:F + 1 - 128, 2 * c + 1, xsl],
                                 emov1[:, bi, :], start=False, stop=True)
            # one merged copy per block: psum [128, 4, 2, 256] -> osb [t, c]
            eng = nc.vector if (bi * 2 + xt_i) % 2 == 0 else nc.scalar
            if eng is nc.vector:
                nc.vector.tensor_copy(
                    osb.rearrange("p t (chi clo) -> p chi clo t", clo=2),
                    pz.rearrange("p m (clo t) -> p m clo t", clo=2))
            else:
                nc.scalar.copy(
                    osb.rearrange("p t (chi clo) -> p chi clo t", clo=2),
                    pz.rearrange("p m (clo t) -> p m clo t", clo=2))
            nc.sync.dma_start(out=out_d[bi, xt_i * 128:(xt_i + 1) * 128, :, :], in_=osb)
    ssbpool.release()
    biapool.release()
    latew.release()
    ht0pool.release()
    sepool.release()
    endw.release()
    ppool.release()
    const.release()


# ------------------------------------------------------------------ entry point

TRACE = False          # set True (from test harnesses) to capture an NTFF profile
LAST_RESULT = None     # BassKernelResults of the most recent run


def kernel(**inputs) -> np.ndarray:
    global LAST_RESULT
    if "nc" not in _CACHE:
        _CACHE["nc"] = _build_program()
    nc = _CACHE["nc"]
    in_maps = _pack_inputs(inputs)
    try:
        res = run_bass_kernel_spmd(nc, in_maps, core_ids=list(range(NCORES)),
                                   trace=TRACE)
    except ModuleNotFoundError:
        # no NTFF profile hook in this container; run without tracing
        res = run_bass_kernel_spmd(nc, in_maps, core_ids=list(range(NCORES)))
    LAST_RESULT = res
    out = np.concatenate([np.asarray(res.results[c]["out"]) for c in range(NCORES)],
                         axis=0)
    return np.ascontiguousarray(out.astype(np.float32))


if __name__ == "__main__":
    raise SystemExit("use test.py")


# revision 42
# speedup vs baseline: 1.0649x; 1.0044x over previous
"""Biaffine NER model (2-layer BiLSTM + highway + biaffine) on 8 Trainium2 cores.

Strategy:
  - Data-parallel over batch: each of the 8 cores handles B_loc=2 of the 16
    batch elements, full model, no collectives.
  - The LSTM recurrences are solved by fixed-point (Jacobi) iteration:
      H^{k+1} = LSTMCell(x_tilde + shift(H^k) @ W_h)
    Each iteration is fully parallel over time (big matmuls, M = B_loc*T = 512
    rows), and the cell-state recurrence c_t = a_t*c_{t-1} + b_t is computed
    with the hardware tensor_tensor_scan. The map contracts by ~4x per
    iteration; K_ITERS=5 sits at ~9.5e-3 rel absmax vs the 2e-2 gate.
  - Everything on-chip is kept "transposed" (feature-major, [128-partition
    folds, (b, t) free]) so matmuls, activations and scans all operate on
    full-width tiles.
  - All contraction dims are zero-padded to multiples of 128 host-side so
    every matmul uses uniform full-128 K-tiles (padding rows are zero on both
    the stationary and moving side).
  - Elementwise gate math runs in bf16 (DVE 2x/4x perf modes).
  - Biases ride as an extra contraction row (ones rail in the moving operand,
    bias row in the stationary operand).
  - Output is DMA'd as bf16 and upcast host-side (halves the output-write
    tail; adds <4e-4 rel err).
"""

import sys

sys.path.insert(0, "/opt/trn_rl_repo")

import ml_dtypes
import numpy as np

import concourse.bass as bass
import concourse.mybir as mybir
import concourse.tile as tile
from concourse.bass_utils import run_bass_kernel_spmd
from concourse.masks import make_identity

F32 = mybir.dt.float32
BF16 = mybir.dt.bfloat16
FP8 = mybir.dt.float8e4
BF16NP = ml_dtypes.bfloat16
F8NP = ml_dtypes.float8_e4m3
AF = mybir.ActivationFunctionType
ALU = mybir.AluOpType
DR = mybir.MatmulPerfMode.DoubleRow
W8SCALE = 128.0           # fp8 weight pre-scale (e4m3 max-normal is 240)

B, T, D = 16, 256, 768
H, H2, G = 400, 800, 1200
F, C = 150, 8
NCORES = 8
BL = B // NCORES          # 2 batch elements per core
L = BL * T                # 512 (b, t) rows per core
GP = 512                  # per-gate padded stride (3*GP = 1536, 12 M-tiles)
NM = 12                   # M-tiles of the padded gate dim
NKH = 4                   # K-tiles of the padded [H+1->512] contraction
NKD = 6                   # K-tiles of D=768
K_ITERS = 5

_CACHE = {}


# ------------------------------------------------------------------ host packing

def _pack_gate_cols(w):
    """[K, 3H] -> [K, 3*GP] with each gate's 400 cols padded to 512."""
    k = w.shape[0]
    out = np.zeros((k, 3 * GP), np.float32)
    for g in range(3):
        out[:, g * GP:g * GP + H] = w[:, g * H:(g + 1) * H]
    return out


def _with_bias_row(w, bias):
    """Append one row (the bias, packed like w's columns) to w."""
    return np.concatenate([w, bias[None, :]], 0)


def _fold_k(w, nk):
    """[K<=128*nk, C] -> [128, nk, C] zero-padded row fold (row r -> [r%128, r//128])."""
    k, c = w.shape
    out = np.zeros((128 * nk, c), np.float32)
    out[:k] = w
    return np.ascontiguousarray(out.reshape(nk, 128, c).transpose(1, 0, 2))


def _fold128(v, nchunk):
    """[n] -> [128, nchunk] column-major fold (unit u -> [u%128, u//128])."""
    out = np.zeros((128, nchunk), np.float32)
    n = len(v)
    for m in range(nchunk):
        seg = v[m * 128:min((m + 1) * 128, n)]
        out[:len(seg), m] = seg
    return out


def _pack_inputs(inputs):
    """Pack weights into the DRAM layouts the program expects (shared by all cores)."""
    f32 = lambda a: np.ascontiguousarray(np.asarray(a, np.float32))
    x = f32(inputs["x"])
    z = np.zeros((3 * GP,), np.float32)

    packs = {}
    fp8packs = {}

    def _fp8_pairs(whfold):
        """[128, 4, C] bf-side fold -> [128, 2(pair), 2(slot), C] fp8, x128."""
        w8 = np.clip(whfold * W8SCALE, -240.0, 240.0).astype(F8NP)
        return np.ascontiguousarray(w8.reshape(128, 2, 2, -1))

    # layer 0: W [D+H, 3H].  The gate bias rides as the last row of wh (it is
    # re-added every Jacobi iteration through the ones slot of ht).
    for nm, wn, bn in (("0f", "W_f0", "b_f0"), ("0b", "W_b0", "b_b0")):
        W = f32(inputs[wn]); bias = _pack_gate_cols(f32(inputs[bn])[None, :])[0]
        packs["wx" + nm] = _fold_k(_pack_gate_cols(W[:D]), NKD)
        wh = _fold_k(_with_bias_row(_pack_gate_cols(W[D:]), bias), NKH)
        packs["wh" + nm] = wh
        fp8packs["wh" + nm + "8"] = _fp8_pairs(wh)
    # layer 1: W [2H+H, 3H]; the input half splits into hf/hb parts (both with
    # zero bias rows -- the bias lives only in wh).
    for nm, wn, bn in (("1f", "W_f1", "b_f1"), ("1b", "W_b1", "b_b1")):
        W = f32(inputs[wn]); bias = _pack_gate_cols(f32(inputs[bn])[None, :])[0]
        packs["wx" + nm + "f"] = _fold_k(_with_bias_row(_pack_gate_cols(W[:H]), z), NKH)
        packs["wx" + nm + "b"] = _fold_k(_with_bias_row(_pack_gate_cols(W[H:H2]), z), NKH)
        wh = _fold_k(_with_bias_row(_pack_gate_cols(W[H2:]), bias), NKH)
        packs["wh" + nm] = wh
        fp8packs["wh" + nm + "8"] = _fp8_pairs(wh)

    # highway: W_hw [2H, 2H]; M packed as [f-half pad 512 | b-half pad 512]
    Whw = f32(inputs["W_hw"]); bhw = f32(inputs["b_hw"])

    def _pack_hw_cols(w):
        k = w.shape[0]
        out = np.zeros((k, 2 * GP), np.float32)
        out[:, 0:H] = w[:, 0:H]
        out[:, GP:GP + H] = w[:, H:H2]
        return out

    zh = np.zeros((2 * GP,), np.float32)
    packs["whwf"] = _fold_k(_with_bias_row(_pack_hw_cols(Whw[:H]), _pack_hw_cols(bhw[None, :])[0]), NKH)
    packs["whwb"] = _fold_k(_with_bias_row(_pack_hw_cols(Whw[H:]), zh), NKH)

    # projections: Ws/We [2H, F]
    for nm, wn, bn in (("s", "W_s", "b_s"), ("e", "W_e", "b_e")):
        W = f32(inputs[wn]); bias = f32(inputs[bn])
        packs["w" + nm + "f"] = _fold_k(_with_bias_row(W[:H], bias), NKH)
        packs["w" + nm + "b"] = _fold_k(_with_bias_row(W[H:], np.zeros((F,), np.float32)), NKH)

    # biaffine U [F+1, C, F+1] -> [F+1, C*256] (each c padded 151->256)
    U = f32(inputs["U"])
    upk = np.zeros((F + 1, C * 256), np.float32)
    for c in range(C):
        upk[:, c * 256:c * 256 + F + 1] = U[:, c, :]
    packs["upk"] = _fold_k(upk, 2)

    packs = {k: v.astype(BF16NP) for k, v in packs.items()}
    packs.update(fp8packs)
    h0f = _fold128(f32(inputs["h0"])[0], 4)
    hti = np.zeros((128, 4, BL, T + 1), np.float32)
    hti[:, :, :, 0] = h0f[:, :, None]          # slot 0 = h0
    hti[16, 3, :, :] = 1.0                     # ones rail for the bias rows
    packs["hti"] = hti.astype(BF16NP)
    packs["hti8"] = hti.astype(F8NP)
    packs["c0f"] = _fold128(f32(inputs["c0"])[0], 4)

    # per-core x, feature-major [128, 6, L]
    per_core = []
    for c in range(NCORES):
        sl = x[c * BL:(c + 1) * BL]
        m = dict(packs)
        m["xT"] = _fold_k(sl.transpose(2, 0, 1).reshape(D, L), NKD).astype(BF16NP)
        per_core.append(m)
    return per_core


# ------------------------------------------------------------------ program

def _build_program():
    nc = bass.Bass(trn_type="TRN2", target_bir_lowering=False, debug=False)

    dins = {}

    def din(name, shape, dt=BF16):
        dins[name] = nc.dram_tensor(name, list(shape), dt, kind="ExternalInput").ap()
        return dins[name]

    din("xT", (128, NKD, L))
    din("wx0f", (128, NKD, 3 * GP)); din("wx0b", (128, NKD, 3 * GP))
    din("wh0f", (128, NKH, 3 * GP)); din("wh0b", (128, NKH, 3 * GP))
    for s in ("1f", "1b"):
        din("wx" + s + "f", (128, NKH, 3 * GP))
        din("wx" + s + "b", (128, NKH, 3 * GP))
        din("wh" + s, (128, NKH, 3 * GP))
    for s in ("0f", "0b", "1f", "1b"):
        din("wh" + s + "8", (128, 2, 2, 3 * GP), dt=FP8)
    din("whwf", (128, NKH, 2 * GP)); din("whwb", (128, NKH, 2 * GP))
    din("wsf", (128, NKH, F)); din("wsb", (128, NKH, F))
    din("wef", (128, NKH, F)); din("web", (128, NKH, F))
    din("upk", (128, 2, C * 256))
    din("hti", (128, 4, BL, T + 1)); din("hti8", (128, 4, BL, T + 1), dt=FP8)
    din("c0f", (128, 4), dt=F32)
    out_d = nc.dram_tensor("out", [BL, T, T, C], BF16, kind="ExternalOutput").ap()

    with tile.TileContext(nc) as tc:
        _body(nc, tc, dins, out_d)
    _split_multi_waits(nc)
    return nc


def _split_multi_waits(nc, max_waits=1):
    """This container's walrus supports only one embedded sync-wait per
    instruction ("Too many sync wait commands"); hoist extra waits onto
    single-wait NoOps inserted just before, on the same engine queue.
    Sequential waiting on monotone semaphores is equivalent to the joint
    wait."""
    n = 0
    for func in nc.m.functions:
        for blk in func.blocks:
            out = []
            for inst in blk.instructions:
                si = inst.sync_info
                if si is not None and si.on_wait and len(si.on_wait) > max_waits:
                    waits = list(si.on_wait)
                    for j, w in enumerate(waits[:-max_waits]):
                        nop = mybir.InstNoOp(name=f"{inst.name}-xw{j}")
                        nop.engine = inst.engine
                        nop.sync_info = mybir.SyncInfo(on_wait=[w], on_update=[])
                        out.append(nop)
                        n += 1
                    inst.sync_info = mybir.SyncInfo(
                        on_wait=waits[-max_waits:], on_update=list(si.on_update))
                out.append(inst)
            blk.instructions = out
    return n


def _load_w(nc, pool, dram, nk, cols, tag, nsplit=1, eng=None):
    """One [128, nk, cols] tile; loaded via `nsplit` DMAs along the k axis."""
    t = pool.tile([128, nk, cols], BF16, name=tag, tag=tag)
    step = (nk + nsplit - 1) // nsplit
    for a in range(0, nk, step):
        b = min(a + step, nk)
        (eng or nc.sync).dma_start(out=t[:, a:b, :], in_=dram[:, a:b, :])
    return t


def _body(nc, tc, dins, out_d):
    # Pool allocation order is the (LIFO) release order, reversed.  Base pools
    # live to the end; big transients nest inside phase windows.
    const = tc.alloc_tile_pool(name="const", bufs=1)
    ppool = tc.alloc_tile_pool(name="psum", bufs=2, space="PSUM")
    endw = tc.alloc_tile_pool(name="endw", bufs=1)        # endgame weights
    sepool = tc.alloc_tile_pool(name="se", bufs=1)        # s1/e1 (+ early ones rows)
    ht0pool = tc.alloc_tile_pool(name="ht0", bufs=1)      # f/br; reused as blend out
    trans = tc.alloc_tile_pool(name="trans", bufs=1)      # released end of phase E
    ht1pool = tc.alloc_tile_pool(name="ht1", bufs=1)      # f/b/br; released end of E
    xtpool = tc.alloc_tile_pool(name="xtilde", bufs=1)    # x~ slots shared by L0/L1
    wh1pool = tc.alloc_tile_pool(name="wh1", bufs=1)      # released end of D
    ht0tmp = tc.alloc_tile_pool(name="ht0tmp", bufs=1)    # b/fr; released end of C

    ident = const.tile([128, 128], BF16)
    make_identity(nc, ident)
    # scaled identity used to inject x~ into the fp8-scaled PSUM groups
    ident128 = const.tile([128, 128], BF16)
    make_identity(nc, ident128)
    nc.vector.tensor_scalar(out=ident128, in0=ident128, scalar1=W8SCALE,
                            scalar2=None, op0=ALU.mult)
    c0sb = const.tile([128, 4], F32)
    nc.sync.dma_start(out=c0sb, in_=dins["c0f"])
    # Engine APs must start at a 32-aligned partition, so "ones" rows living at
    # odd partitions are written via SBUF->SBUF DMA from this partition-0 tile.
    ones_c = const.tile([1, BL, T + 1], BF16)
    nc.vector.memset(ones_c, 1.0)

    def init_ht(ht):
        # fresh-SBUF init in ONE DMA (DMA instructions only support one wait):
        # zeros + h0 at slot 0 + the ones rail for the bias rows.
        nc.sync.dma_start(out=ht, in_=dins["hti"])

    # All recurrence state tensors are allocated and initialized up front, on
    # fresh SBUF, so their init DMAs carry at most one sync wait each (the DMA
    # lowering only supports a single wait condition).
    ht0 = {}
    ht1 = {}
    ht8 = {}
    ht0["f"] = ht0pool.tile([128, 4, BL, T + 1], BF16, name="ht0f", tag="ht0f")
    ht0["br"] = ht0pool.tile([128, 4, BL, T + 1], BF16, name="ht0br", tag="ht0br")
    for s in ("0f", "0b", "1f", "1b"):
        ht8[s] = ht0pool.tile([128, 4, BL, T + 1], FP8, name="ht8" + s, tag="ht8" + s)
    ht0["b"] = ht0tmp.tile([128, 4, BL, T + 1], BF16, name="ht0b", tag="ht0b")
    ht0["fr"] = ht0tmp.tile([128, 4, BL, T + 1], BF16, name="ht0fr", tag="ht0fr")
    ht1["f"] = ht1pool.tile([128, 4, BL, T + 1], BF16, name="ht1f", tag="ht1f")
    ht1["b"] = ht1pool.tile([128, 4, BL, T + 1], BF16, name="ht1b", tag="ht1b")
    ht1["br"] = ht1pool.tile([128, 4, BL, T + 1], BF16, name="ht1br", tag="ht1br")
    wh1 = {}
    wh1_8 = {"f": wh1pool.tile([128, 2, 2, 3 * GP], FP8, name="wh1f8", tag="wh1f8"),
             "b": wh1pool.tile([128, 2, 2, 3 * GP], FP8, name="wh1b8", tag="wh1b8")}

    # -------- phase A: layer-0 x_tilde (feature-major) --------
    whpool = tc.alloc_tile_pool(name="wh0", bufs=1)
    xpool = tc.alloc_tile_pool(name="xt", bufs=1)
    xt_sb = _load_w(nc, xpool, dins["xT"], NKD, L, "xt")
    # time-reversed view of the same tile for the backward stream
    xt_rev = xt_sb.rearrange("p k (b t) -> p k b t", b=BL)[:, :, :, ::-1]
    # f-dir weights next: the first matmul can start as soon as xT + the first
    # wx0f chunk have landed
    wx0f = _load_w(nc, xpool, dins["wx0f"], NKD, 3 * GP, "wx0", nsplit=3)

    # recurrence-state init + phase-B fp8 weights, issued behind the phase-A
    # loads.  Everything needed later (bf16 wh for the final iterations, the
    # layer-1 state inits, the highway weights) is issued from inside the
    # Jacobi windows so it never delays the phase-A/B critical path.
    wh0_8 = {"f": whpool.tile([128, 2, 2, 3 * GP], FP8, name="wh0f8", tag="wh0f8"),
             "b": whpool.tile([128, 2, 2, 3 * GP], FP8, name="wh0b8", tag="wh0b8")}
    nc.sync.dma_start(out=wh0_8["f"], in_=dins["wh0f8"])
    for t_ in (ht0["f"], ht0["b"]):
        init_ht(t_)
    for s in ("0f", "0b"):
        nc.sync.dma_start(out=ht8[s], in_=dins["hti8"])
    nc.sync.dma_start(out=wh0_8["b"], in_=dins["wh0b8"])
    wh0 = {}
    s1T = {}
    for nm in ("s", "e"):
        st = sepool.tile([128, 2, L], BF16, name=nm + "1T", tag=nm + "1T")
        nc.sync.dma_start(out=st[F - 128:F - 127, 1, :],
                          in_=ones_c.rearrange("p b t -> p (b t)")[:, 0:L])
        s1T[nm] = st
    whw = {}

    def deferred_b_loads():
        # issued behind the wx0b reload ON THE SAME (gpsimd) DMA queue, so
        # none of it can jump ahead of wx0b on the shared DMA engines.  The
        # bf16 wh0 is only read by the final (bf16) iteration ~4 iterations
        # later; everything else here is needed from phase C onward.
        g = nc.gpsimd
        wh0["f"] = _load_w(nc, whpool, dins["wh0f"], NKH, 3 * GP, "wh0f", eng=g)
        wh0["b"] = _load_w(nc, whpool, dins["wh0b"], NKH, 3 * GP, "wh0b", eng=g)
        g.dma_start(out=wh1_8["f"], in_=dins["wh1f8"])
        g.dma_start(out=wh1_8["b"], in_=dins["wh1b8"])
        for t_ in (ht1["f"], ht1["b"]):
            g.dma_start(out=t_, in_=dins["hti"])
        for s in ("1f", "1b"):
            g.dma_start(out=ht8[s], in_=dins["hti8"])
        whw["f"] = _load_w(nc, endw, dins["whwf"], NKH, 2 * GP, "whwf", eng=g)
        whw["b"] = _load_w(nc, endw, dins["whwb"], NKH, 2 * GP, "whwb", eng=g)
        wh1["f"] = _load_w(nc, wh1pool, dins["wh1f"], NKH, 3 * GP, "wh1f", eng=g)
        wh1["b"] = _load_w(nc, wh1pool, dins["wh1b"], NKH, 3 * GP, "wh1b", eng=g)

    def psum_tile():
        return ppool.tile([128, 4, GP], F32, name="pz", tag="pz")

    xt0 = {}

    def phase_a_dir(s, wt):
        store = xtpool.tile([128, NM, GP], BF16, name="xt0" + s, tag="xt" + s)
        for grp in range(3):
            pz = psum_tile()
            for mi in range(4):
                m = grp * 4 + mi
                for k in range(NKD):
                    mov = xt_sb[:, k, :] if s == "f" else xt_rev[:, k, :, :]
                    nc.tensor.matmul(pz[:, mi, :], wt[:, k, m * 128:(m + 1) * 128],
                                     mov, start=(k == 0), stop=(k == NKD - 1))
            nc.scalar.copy(store[:, grp * 4:(grp + 1) * 4, :], pz)
        xt0[s] = store

    def jacobi_iter(stream, it):
        """Emit one Jacobi iteration for one direction stream.
        stream = (wh_dict, key, wh_pairs_fp8, xs, ht_bf16, ht_fp8, si);
        ht is [128, 4, 2, 257], pre-initialized (slot 0 = h0, ones rail at
        [16, 3], zeros).

        Iterations 0..K-2 run the recurrence matmuls in fp8 DoubleRow mode
        (2 K-tiles per instruction at 0.5 cycles/row) against the fp8 h-state;
        weights are pre-scaled by W8SCALE and the activation divides it back
        out.  The x~ injection stays bf16 (its accuracy persists into the
        fixed point).  The last iteration runs fully in bf16: the fp8
        quantization noise of earlier iterates contracts by ~4x per iteration,
        so only bf16-level noise survives in the final h."""
        wh_d, wh_k, wh_p8, xs, ht, h8, si = stream
        fp8 = it < K_ITERS - 1
        mov = h8 if fp8 else ht
        # the last fp8 iteration feeds the bf16 one: write bf16 h
        wout = ht if it >= K_ITERS - 2 else h8
        ss = str(si)
        I = trans.tile([128, 4, BL, T], BF16, name="I" + ss, tag="I" + ss)
        Gt = trans.tile([128, 4, BL, T], BF16, name="Gt" + ss, tag="Gt" + ss)
        O = trans.tile([128, 4, BL, T], BF16, name="O" + ss, tag="O" + ss)
        for g, (dst, fn) in enumerate(((I, AF.Sigmoid), (Gt, AF.Tanh),
                                       (O, AF.Sigmoid))):
            pz = psum_tile()
            for mi in range(4):
                m = g * 4 + mi
                nc.tensor.matmul(pz[:, mi, :], ident128 if fp8 else ident,
                                 xs[:, m, :], start=True, stop=False)
                if fp8:
                    for pair in range(2):
                        nc.tensor.matmul(
                            pz[:, mi, :],
                            wh_p8[:, pair, :, m * 128:(m + 1) * 128],
                            mov[:, 2 * pair:2 * pair + 2, :, 0:T],
                            start=False, stop=(pair == 1), perf_mode=DR)
                else:
                    for k in range(NKH):
                        nc.tensor.matmul(pz[:, mi, :],
                                         wh_d[wh_k][:, k, m * 128:(m + 1) * 128],
                                         mov[:, k, :, 0:T],
                                         start=False, stop=(k == NKH - 1))
            nc.scalar.activation(dst, pz.rearrange("p m (b t) -> p m b t", b=BL),
                                 fn, scale=(1.0 / W8SCALE) if fp8 else 1.0)
        nc.vector.tensor_mul(Gt, I, Gt)                     # b_t = i * g
        nc.vector.tensor_scalar(out=I, in0=I, scalar1=-1.0, scalar2=1.0,
                                op0=ALU.mult, op1=ALU.add)
        for k in range(4):                                   # c scan per (chunk, b)
            for b in range(BL):
                # in-place over Gt: the scan reads data1[t] before
                # writing out[t], so out may alias data1
                nc.vector.tensor_tensor_scan(
                    out=Gt[:, k, b, :], data0=I[:, k, b, :], data1=Gt[:, k, b, :],
                    initial=c0sb[:, k:k + 1],
                    op0=ALU.mult, op1=ALU.add)
        nc.scalar.activation(Gt, Gt, AF.Tanh)
        nc.vector.tensor_mul(wout[:, 0:3, :, 1:T + 1], Gt[:, 0:3], O[:, 0:3])
        nc.vector.tensor_mul(wout[0:16, 3, :, 1:T + 1], Gt[0:16, 3], O[0:16, 3])

    def lstm_jacobi_pair(streams, skip=()):
        for it in range(K_ITERS):
            for stream in streams:
                if (stream[-1], it) not in skip:
                    jacobi_iter(stream, it)

    # -------- phases A+B interleaved --------
    # A-f; then Jacobi-L0 f iteration 0 keeps the PE busy while the wx0b
    # reload (WAR on the shared wx0 buffer) and the deferred weight loads
    # drain on the gpsimd DMA queue; then A-b; then the remaining iterations.
    streamBf = (wh0, "f", wh0_8["f"], xt0, ht0["f"], ht8["0f"], 0)
    streamBb = (wh0, "b", wh0_8["b"], xt0, ht0["b"], ht8["0b"], 1)
    phase_a_dir("f", wx0f)
    streamBf = streamBf[:3] + (xt0["f"],) + streamBf[4:]
    jacobi_iter(streamBf, 0)
    wx0b = _load_w(nc, xpool, dins["wx0b"], NKD, 3 * GP, "wx0", nsplit=3,
                   eng=nc.gpsimd)
    deferred_b_loads()
    phase_a_dir("b", wx0b)
    xpool.release()
    streamBb = streamBb[:3] + (xt0["b"],) + streamBb[4:]
    lstm_jacobi_pair([streamBf, streamBb], skip={(0, 0)})
    whpool.release()

    # reversed-time copies (the ones rail at [16, 3] copies over too)
    nc.vector.tensor_copy(ht0["fr"][:, :, :, 1:T + 1], ht0["f"][:, :, :, T:0:-1])
    nc.vector.tensor_copy(ht0["br"][:, :, :, 1:T + 1], ht0["b"][:, :, :, T:0:-1])

    # -------- phase C: layer-1 x_tilde --------
    wx1pool = tc.alloc_tile_pool(name="wx1", bufs=1)
    wx1 = {}
    for s in ("f", "b"):
        wx1[s] = (_load_w(nc, wx1pool, dins["wx1" + s + "f"], NKH, 3 * GP,
                          "wx1" + s + "f", eng=nc.gpsimd),
                  _load_w(nc, wx1pool, dins["wx1" + s + "b"], NKH, 3 * GP,
                          "wx1" + s + "b", eng=nc.gpsimd))

    xt1 = {}
    for s, (hfmov, hbmov) in (("f", (ht0["f"], ht0["br"])), ("b", (ht0["fr"], ht0["b"]))):
        wtf, wtb = wx1[s]
        store = xtpool.tile([128, NM, GP], BF16, name="xt1" + s, tag="xt" + s)
        pairs = [(wtf, hfmov, k) for k in range(NKH)] + [(wtb, hbmov, k) for k in range(NKH)]
        for grp in range(3):
            pz = psum_tile()
            for mi in range(4):
                m = grp * 4 + mi
                for pi, (wt, mov, k) in enumerate(pairs):
                    nc.tensor.matmul(pz[:, mi, :], wt[:, k, m * 128:(m + 1) * 128],
                                     mov[:, k, :, 1:T + 1],
                                     start=(pi == 0), stop=(pi == 7))
            nc.scalar.copy(store[:, grp * 4:(grp + 1) * 4, :], pz)
        xt1[s] = store
    wx1pool.release()
    ht0tmp.release()

    # -------- phase D: layer-1 recurrences (both directions interleaved) -----
    lstm_jacobi_pair([(wh1, "f", wh1_8["f"], xt1["f"], ht1["f"], ht8["1f"], 0),
                      (wh1, "b", wh1_8["b"], xt1["b"], ht1["b"], ht8["1b"], 1)])
    wh1pool.release()
    nc.vector.tensor_copy(ht1["br"][:, :, :, 1:T + 1], ht1["b"][:, :, :, T:0:-1])
    xtpool.release()

    # -------- phase E: highway gate + blend (in place over ht0 f/br slots) ----
    outT = {}
    pairs = [(whw["f"], ht1["f"], k) for k in range(NKH)] + \
            [(whw["b"], ht1["br"], k) for k in range(NKH)]
    for half, (h1, h0) in (("f", (ht1["f"], ht0["f"])), ("b", (ht1["br"], ht0["br"]))):
        pz = psum_tile()
        for mi in range(4):
            m = (0 if half == "f" else 4) + mi
            for pi, (wt, mov, k) in enumerate(pairs):
                nc.tensor.matmul(pz[:, mi, :], wt[:, k, m * 128:(m + 1) * 128],
                                 mov[:, k, :, 1:T + 1],
                                 start=(pi == 0), stop=(pi == 7))
        gate = trans.tile([128, 4, BL, T], BF16, name="gate", tag="I0")
        nc.scalar.activation(gate, pz.rearrange("p m (b t) -> p m b t", b=BL), AF.Sigmoid)
        tmp = trans.tile([128, 4, BL, T], BF16, name="tmpb", tag="Gt0")
        hsl = h0[:, :, :, 1:T + 1]
        nc.vector.tensor_sub(tmp, h1[:, :, :, 1:T + 1], hsl)
        nc.vector.tensor_mul(tmp, gate, tmp)
        # the final write skips partition 16 of chunk 3 so the ones rail from
        # the init image survives for the projection bias rows
        nc.vector.tensor_add(hsl[:, 0:3], hsl[:, 0:3], tmp[:, 0:3])
        nc.vector.tensor_add(hsl[0:16, 3], hsl[0:16, 3], tmp[0:16, 3])
        outT[half] = h0
    ht1pool.release()
    trans.release()

    # -------- phase F: s/e projections --------
    latew = tc.alloc_tile_pool(name="latew", bufs=1)
    wse = {}
    for nm in ("s", "e"):
        wse[nm] = {"f": _load_w(nc, latew, dins["w" + nm + "f"], NKH, F, "w" + nm + "f"),
                   "b": _load_w(nc, latew, dins["w" + nm + "b"], NKH, F, "w" + nm + "b")}
    ut = _load_w(nc, latew, dins["upk"], 2, C * 256, "upk")
    for nm in ("s", "e"):
        wf, wb = wse[nm]["f"], wse[nm]["b"]
        st = s1T[nm]
        prs = [(wf, outT["f"], k) for k in range(NKH)] + [(wb, outT["b"], k) for k in range(NKH)]
        pz = psum_tile()
        for mi, (ma, mb) in enumerate(((0, 128), (128, F))):
            for pi, (wt, mov, k) in enumerate(prs):
                nc.tensor.matmul(pz[0:mb - ma, mi, :], wt[:, k, ma:mb],
                                 mov[:, k, :, 1:T + 1],
                                 start=(pi == 0), stop=(pi == 7))
        nc.scalar.copy(st[:, 0, :], pz[:, 0, :])
        nc.scalar.copy(st[0:F - 128, 1, :], pz[0:F - 128, 1, :])

    # -------- phase G: biaffine part 1: tmp[(c,j), (b,t)] --------
    biapool = tc.alloc_tile_pool(name="bia", bufs=1)
    smov = [s1T["s"][:, 0, :], s1T["s"][0:F + 1 - 128, 1, :]]
    ut_t = [ut[:, 0, :], ut[0:F + 1 - 128, 1, :]]
    tmpT = biapool.tile([128, 16, GP], BF16, name="tmpT", tag="tmpT")
    for grp in range(4):
        pz = psum_tile()
        for mi in range(4):
            m = grp * 4 + mi
            for k in range(2):
                nc.tensor.matmul(pz[:, mi, :], ut_t[k][:, m * 128:(m + 1) * 128],
                                 smov[k], start=(k == 0), stop=(k == 1))
        nc.scalar.copy(tmpT[:, grp * 4:(grp + 1) * 4, :], pz)

    # -------- phase H: biaffine part 2 + output assembly --------
    emov0 = s1T["e"][:, 0, :].rearrange("p (b t) -> p b t", b=BL)
    emov1 = s1T["e"][0:F + 1 - 128, 1, :].rearrange("p (b t) -> p b t", b=BL)
    ssbpool = tc.alloc_tile_pool(name="osb", bufs=2)
    for bi in range(BL):
        for xt_i in range(2):
            osb = ssbpool.tile([128, T, C], BF16, name="osb", tag="osb")
            pz = psum_tile()
            for c in range(C):
                xsl = slice(bi * T + xt_i * 128, bi * T + xt_i * 128 + 128)
                po = pz[:, c // 2, (c % 2) * T:(c % 2) * T + T]
                nc.tensor.matmul(po, tmpT[:, 2 * c, xsl], emov0[:, bi, :],
                                 start=True, stop=False)
                nc.tensor.matmul(po, tmpT[0:F + 1 - 128, 2 * c + 1, xsl],
                                 emov1[:, bi, :], start=False, stop=True)
            # one merged copy per block: psum [128, 4, 2, 256] -> osb [t, c]
            eng = nc.vector if (bi * 2 + xt_i) % 2 == 0 else nc.scalar
            if eng is nc.vector:
                nc.vector.tensor_copy(
                    osb.rearrange("p t (chi clo) -> p chi clo t", clo=2),
                    pz.rearrange("p m (clo t) -> p m clo t", clo=2))
            else:
                nc.scalar.copy(
                    osb.rearrange("p t (chi clo) -> p chi clo t", clo=2),
                    pz.rearrange("p m (clo t) -> p m clo t", clo=2))
            nc.sync.dma_start(out=out_d[bi, xt_i * 128:(xt_i + 1) * 128, :, :], in_=osb)
    ssbpool.release()
    biapool.release()
    latew.release()
    ht0pool.release()
    sepool.release()
    endw.release()
    ppool.release()
    const.release()


# ------------------------------------------------------------------ entry point

TRACE = False          # set True (from test harnesses) to capture an NTFF profile
LAST_RESULT = None     # BassKernelResults of the most recent run


def kernel(**inputs) -> np.ndarray:
    global LAST_RESULT
    if "nc" not in _CACHE:
        _CACHE["nc"] = _build_program()
    nc = _CACHE["nc"]
    in_maps = _pack_inputs(inputs)
    try:
        res = run_bass_kernel_spmd(nc, in_maps, core_ids=list(range(NCORES)),
                                   trace=TRACE)
    except ModuleNotFoundError:
        # no NTFF profile hook in this container; run without tracing
        res = run_bass_kernel_spmd(nc, in_maps, core_ids=list(range(NCORES)))
    LAST_RESULT = res
    out = np.concatenate([np.asarray(res.results[c]["out"]) for c in range(NCORES)],
                         axis=0)
    return np.ascontiguousarray(out.astype(np.float32))


if __name__ == "__main__":
    raise SystemExit("use test.py")


# revision 45
# speedup vs baseline: 1.0664x; 1.0014x over previous
"""Biaffine NER model (2-layer BiLSTM + highway + biaffine) on 8 Trainium2 cores.

Strategy:
  - Data-parallel over batch: each of the 8 cores handles B_loc=2 of the 16
    batch elements, full model, no collectives.
  - The LSTM recurrences are solved by fixed-point (Jacobi) iteration:
      H^{k+1} = LSTMCell(x_tilde + shift(H^k) @ W_h)
    Each iteration is fully parallel over time (big matmuls, M = B_loc*T = 512
    rows), and the cell-state recurrence c_t = a_t*c_{t-1} + b_t is computed
    with the hardware tensor_tensor_scan. The map contracts by ~4x per
    iteration; K_ITERS=5 sits at ~9.5e-3 rel absmax vs the 2e-2 gate.
  - Everything on-chip is kept "transposed" (feature-major, [128-partition
    folds, (b, t) free]) so matmuls, activations and scans all operate on
    full-width tiles.
  - All contraction dims are zero-padded to multiples of 128 host-side so
    every matmul uses uniform full-128 K-tiles (padding rows are zero on both
    the stationary and moving side).
  - Elementwise gate math runs in bf16 (DVE 2x/4x perf modes).
  - Biases ride as an extra contraction row (ones rail in the moving operand,
    bias row in the stationary operand).
  - Output is DMA'd as bf16 and upcast host-side (halves the output-write
    tail; adds <4e-4 rel err).
"""

import sys

sys.path.insert(0, "/opt/trn_rl_repo")

import ml_dtypes
import numpy as np

import concourse.bass as bass
import concourse.mybir as mybir
import concourse.tile as tile
from concourse.bass_utils import run_bass_kernel_spmd
from concourse.masks import make_identity

F32 = mybir.dt.float32
BF16 = mybir.dt.bfloat16
FP8 = mybir.dt.float8e4
BF16NP = ml_dtypes.bfloat16
F8NP = ml_dtypes.float8_e4m3
AF = mybir.ActivationFunctionType
ALU = mybir.AluOpType
DR = mybir.MatmulPerfMode.DoubleRow
W8SCALE = 128.0           # fp8 weight pre-scale (e4m3 max-normal is 240)

B, T, D = 16, 256, 768
H, H2, G = 400, 800, 1200
F, C = 150, 8
NCORES = 8
BL = B // NCORES          # 2 batch elements per core
L = BL * T                # 512 (b, t) rows per core
GP = 512                  # per-gate padded stride (3*GP = 1536, 12 M-tiles)
NM = 12                   # M-tiles of the padded gate dim
NKH = 4                   # K-tiles of the padded [H+1->512] contraction
NKD = 6                   # K-tiles of D=768
K_ITERS = 5

_CACHE = {}


# ------------------------------------------------------------------ host packing

def _pack_gate_cols(w):
    """[K, 3H] -> [K, 3*GP] with each gate's 400 cols padded to 512."""
    k = w.shape[0]
    out = np.zeros((k, 3 * GP), np.float32)
    for g in range(3):
        out[:, g * GP:g * GP + H] = w[:, g * H:(g + 1) * H]
    return out


def _with_bias_row(w, bias):
    """Append one row (the bias, packed like w's columns) to w."""
    return np.concatenate([w, bias[None, :]], 0)


def _fold_k(w, nk):
    """[K<=128*nk, C] -> [128, nk, C] zero-padded row fold (row r -> [r%128, r//128])."""
    k, c = w.shape
    out = np.zeros((128 * nk, c), np.float32)
    out[:k] = w
    return np.ascontiguousarray(out.reshape(nk, 128, c).transpose(1, 0, 2))


def _fold128(v, nchunk):
    """[n] -> [128, nchunk] column-major fold (unit u -> [u%128, u//128])."""
    out = np.zeros((128, nchunk), np.float32)
    n = len(v)
    for m in range(nchunk):
        seg = v[m * 128:min((m + 1) * 128, n)]
        out[:len(seg), m] = seg
    return out


def _pack_inputs(inputs):
    """Pack weights into the DRAM layouts the program expects (shared by all cores)."""
    f32 = lambda a: np.ascontiguousarray(np.asarray(a, np.float32))
    x = f32(inputs["x"])
    z = np.zeros((3 * GP,), np.float32)

    packs = {}
    fp8packs = {}

    def _fp8_pairs(whfold):
        """[128, 4, C] bf-side fold -> [128, 2(pair), 2(slot), C] fp8, x128."""
        w8 = np.clip(whfold * W8SCALE, -240.0, 240.0).astype(F8NP)
        return np.ascontiguousarray(w8.reshape(128, 2, 2, -1))

    # layer 0: W [D+H, 3H].  The gate bias rides as the last row of wh (it is
    # re-added every Jacobi iteration through the ones slot of ht).
    for nm, wn, bn in (("0f", "W_f0", "b_f0"), ("0b", "W_b0", "b_b0")):
        W = f32(inputs[wn]); bias = _pack_gate_cols(f32(inputs[bn])[None, :])[0]
        packs["wx" + nm] = _fold_k(_pack_gate_cols(W[:D]), NKD)
        wh = _fold_k(_with_bias_row(_pack_gate_cols(W[D:]), bias), NKH)
        packs["wh" + nm] = wh
        fp8packs["wh" + nm + "8"] = _fp8_pairs(wh)
    # layer 1: W [2H+H, 3H]; the input half splits into hf/hb parts (both with
    # zero bias rows -- the bias lives only in wh).
    for nm, wn, bn in (("1f", "W_f1", "b_f1"), ("1b", "W_b1", "b_b1")):
        W = f32(inputs[wn]); bias = _pack_gate_cols(f32(inputs[bn])[None, :])[0]
        packs["wx" + nm + "f"] = _fold_k(_with_bias_row(_pack_gate_cols(W[:H]), z), NKH)
        packs["wx" + nm + "b"] = _fold_k(_with_bias_row(_pack_gate_cols(W[H:H2]), z), NKH)
        wh = _fold_k(_with_bias_row(_pack_gate_cols(W[H2:]), bias), NKH)
        packs["wh" + nm] = wh
        fp8packs["wh" + nm + "8"] = _fp8_pairs(wh)

    # highway: W_hw [2H, 2H]; M packed as [f-half pad 512 | b-half pad 512]
    Whw = f32(inputs["W_hw"]); bhw = f32(inputs["b_hw"])

    def _pack_hw_cols(w):
        k = w.shape[0]
        out = np.zeros((k, 2 * GP), np.float32)
        out[:, 0:H] = w[:, 0:H]
        out[:, GP:GP + H] = w[:, H:H2]
        return out

    zh = np.zeros((2 * GP,), np.float32)
    packs["whwf"] = _fold_k(_with_bias_row(_pack_hw_cols(Whw[:H]), _pack_hw_cols(bhw[None, :])[0]), NKH)
    packs["whwb"] = _fold_k(_with_bias_row(_pack_hw_cols(Whw[H:]), zh), NKH)

    # projections: Ws/We [2H, F]
    for nm, wn, bn in (("s", "W_s", "b_s"), ("e", "W_e", "b_e")):
        W = f32(inputs[wn]); bias = f32(inputs[bn])
        packs["w" + nm + "f"] = _fold_k(_with_bias_row(W[:H], bias), NKH)
        packs["w" + nm + "b"] = _fold_k(_with_bias_row(W[H:], np.zeros((F,), np.float32)), NKH)

    # biaffine U [F+1, C, F+1] -> [F+1, C*256] (each c padded 151->256)
    U = f32(inputs["U"])
    upk = np.zeros((F + 1, C * 256), np.float32)
    for c in range(C):
        upk[:, c * 256:c * 256 + F + 1] = U[:, c, :]
    packs["upk"] = _fold_k(upk, 2)

    packs = {k: v.astype(BF16NP) for k, v in packs.items()}
    packs.update(fp8packs)
    h0f = _fold128(f32(inputs["h0"])[0], 4)
    hti = np.zeros((128, 4, BL, T + 1), np.float32)
    hti[:, :, :, 0] = h0f[:, :, None]          # slot 0 = h0
    hti[16, 3, :, :] = 1.0                     # ones rail for the bias rows
    packs["hti"] = hti.astype(BF16NP)
    packs["hti8"] = hti.astype(F8NP)
    packs["c0f"] = _fold128(f32(inputs["c0"])[0], 4)

    # per-core x, feature-major [128, 6, L]
    per_core = []
    for c in range(NCORES):
        sl = x[c * BL:(c + 1) * BL]
        m = dict(packs)
        m["xT"] = _fold_k(sl.transpose(2, 0, 1).reshape(D, L), NKD).astype(BF16NP)
        per_core.append(m)
    return per_core


# ------------------------------------------------------------------ program

def _build_program():
    nc = bass.Bass(trn_type="TRN2", target_bir_lowering=False, debug=False)

    dins = {}

    def din(name, shape, dt=BF16):
        dins[name] = nc.dram_tensor(name, list(shape), dt, kind="ExternalInput").ap()
        return dins[name]

    din("xT", (128, NKD, L))
    din("wx0f", (128, NKD, 3 * GP)); din("wx0b", (128, NKD, 3 * GP))
    din("wh0f", (128, NKH, 3 * GP)); din("wh0b", (128, NKH, 3 * GP))
    for s in ("1f", "1b"):
        din("wx" + s + "f", (128, NKH, 3 * GP))
        din("wx" + s + "b", (128, NKH, 3 * GP))
        din("wh" + s, (128, NKH, 3 * GP))
    for s in ("0f", "0b", "1f", "1b"):
        din("wh" + s + "8", (128, 2, 2, 3 * GP), dt=FP8)
    din("whwf", (128, NKH, 2 * GP)); din("whwb", (128, NKH, 2 * GP))
    din("wsf", (128, NKH, F)); din("wsb", (128, NKH, F))
    din("wef", (128, NKH, F)); din("web", (128, NKH, F))
    din("upk", (128, 2, C * 256))
    din("hti", (128, 4, BL, T + 1)); din("hti8", (128, 4, BL, T + 1), dt=FP8)
    din("c0f", (128, 4), dt=F32)
    out_d = nc.dram_tensor("out", [BL, T, T, C], BF16, kind="ExternalOutput").ap()

    with tile.TileContext(nc) as tc:
        _body(nc, tc, dins, out_d)
    _split_multi_waits(nc)
    return nc


def _split_multi_waits(nc, max_waits=1):
    """This container's walrus supports only one embedded sync-wait per
    instruction ("Too many sync wait commands"); hoist extra waits onto
    single-wait NoOps inserted just before, on the same engine queue.
    Sequential waiting on monotone semaphores is equivalent to the joint
    wait."""
    n = 0
    for func in nc.m.functions:
        for blk in func.blocks:
            out = []
            for inst in blk.instructions:
                si = inst.sync_info
                if si is not None and si.on_wait and len(si.on_wait) > max_waits:
                    waits = list(si.on_wait)
                    for j, w in enumerate(waits[:-max_waits]):
                        nop = mybir.InstNoOp(name=f"{inst.name}-xw{j}")
                        nop.engine = inst.engine
                        nop.sync_info = mybir.SyncInfo(on_wait=[w], on_update=[])
                        out.append(nop)
                        n += 1
                    inst.sync_info = mybir.SyncInfo(
                        on_wait=waits[-max_waits:], on_update=list(si.on_update))
                out.append(inst)
            blk.instructions = out
    return n


def _load_w(nc, pool, dram, nk, cols, tag, nsplit=1, eng=None):
    """One [128, nk, cols] tile; loaded via `nsplit` DMAs along the k axis."""
    t = pool.tile([128, nk, cols], BF16, name=tag, tag=tag)
    step = (nk + nsplit - 1) // nsplit
    for a in range(0, nk, step):
        b = min(a + step, nk)
        (eng or nc.sync).dma_start(out=t[:, a:b, :], in_=dram[:, a:b, :])
    return t


def _body(nc, tc, dins, out_d):
    # Pool allocation order is the (LIFO) release order, reversed.  Base pools
    # live to the end; big transients nest inside phase windows.
    const = tc.alloc_tile_pool(name="const", bufs=1)
    ppool = tc.alloc_tile_pool(name="psum", bufs=2, space="PSUM")
    endw = tc.alloc_tile_pool(name="endw", bufs=1)        # endgame weights
    sepool = tc.alloc_tile_pool(name="se", bufs=1)        # s1/e1 (+ early ones rows)
    ht0pool = tc.alloc_tile_pool(name="ht0", bufs=1)      # f/br; reused as blend out
    trans = tc.alloc_tile_pool(name="trans", bufs=1)      # released end of phase E
    ht1pool = tc.alloc_tile_pool(name="ht1", bufs=1)      # f/b/br; released end of E
    xtpool = tc.alloc_tile_pool(name="xtilde", bufs=1)    # x~ slots shared by L0/L1
    wh1pool = tc.alloc_tile_pool(name="wh1", bufs=1)      # released end of D
    ht0tmp = tc.alloc_tile_pool(name="ht0tmp", bufs=1)    # b/fr; released end of C

    ident = const.tile([128, 128], BF16)
    make_identity(nc, ident)
    # scaled identity used to inject x~ into the fp8-scaled PSUM groups
    ident128 = const.tile([128, 128], BF16)
    make_identity(nc, ident128)
    nc.vector.tensor_scalar(out=ident128, in0=ident128, scalar1=W8SCALE,
                            scalar2=None, op0=ALU.mult)
    c0sb = const.tile([128, 4], F32)
    nc.sync.dma_start(out=c0sb, in_=dins["c0f"])
    # Engine APs must start at a 32-aligned partition, so "ones" rows living at
    # odd partitions are written via SBUF->SBUF DMA from this partition-0 tile.
    ones_c = const.tile([1, BL, T + 1], BF16)
    nc.vector.memset(ones_c, 1.0)

    def init_ht(ht):
        # fresh-SBUF init in ONE DMA (DMA instructions only support one wait):
        # zeros + h0 at slot 0 + the ones rail for the bias rows.
        nc.sync.dma_start(out=ht, in_=dins["hti"])

    # All recurrence state tensors are allocated and initialized up front, on
    # fresh SBUF, so their init DMAs carry at most one sync wait each (the DMA
    # lowering only supports a single wait condition).
    ht0 = {}
    ht1 = {}
    ht8 = {}
    ht0["f"] = ht0pool.tile([128, 4, BL, T + 1], BF16, name="ht0f", tag="ht0f")
    ht0["br"] = ht0pool.tile([128, 4, BL, T + 1], BF16, name="ht0br", tag="ht0br")
    for s in ("0f", "0b", "1f", "1b"):
        ht8[s] = ht0pool.tile([128, 4, BL, T + 1], FP8, name="ht8" + s, tag="ht8" + s)
    ht0["b"] = ht0tmp.tile([128, 4, BL, T + 1], BF16, name="ht0b", tag="ht0b")
    ht0["fr"] = ht0tmp.tile([128, 4, BL, T + 1], BF16, name="ht0fr", tag="ht0fr")
    ht1["f"] = ht1pool.tile([128, 4, BL, T + 1], BF16, name="ht1f", tag="ht1f")
    ht1["b"] = ht1pool.tile([128, 4, BL, T + 1], BF16, name="ht1b", tag="ht1b")
    ht1["br"] = ht1pool.tile([128, 4, BL, T + 1], BF16, name="ht1br", tag="ht1br")
    wh1 = {}
    wh1_8 = {"f": wh1pool.tile([128, 2, 2, 3 * GP], FP8, name="wh1f8", tag="wh1f8"),
             "b": wh1pool.tile([128, 2, 2, 3 * GP], FP8, name="wh1b8", tag="wh1b8")}

    # -------- phase A: layer-0 x_tilde (feature-major) --------
    whpool = tc.alloc_tile_pool(name="wh0", bufs=1)
    xpool = tc.alloc_tile_pool(name="xt", bufs=1)
    xt_sb = _load_w(nc, xpool, dins["xT"], NKD, L, "xt", nsplit=2)
    # time-reversed view of the same tile for the backward stream
    xt_rev = xt_sb.rearrange("p k (b t) -> p k b t", b=BL)[:, :, :, ::-1]
    # f-dir weights next: the first matmul can start as soon as xT + the first
    # wx0f chunk have landed
    wx0f = _load_w(nc, xpool, dins["wx0f"], NKD, 3 * GP, "wx0", nsplit=3)

    # recurrence-state init + phase-B fp8 weights, issued behind the phase-A
    # loads.  Everything needed later (bf16 wh for the final iterations, the
    # layer-1 state inits, the highway weights) is issued from inside the
    # Jacobi windows so it never delays the phase-A/B critical path.
    wh0_8 = {"f": whpool.tile([128, 2, 2, 3 * GP], FP8, name="wh0f8", tag="wh0f8"),
             "b": whpool.tile([128, 2, 2, 3 * GP], FP8, name="wh0b8", tag="wh0b8")}
    nc.sync.dma_start(out=wh0_8["f"], in_=dins["wh0f8"])
    for t_ in (ht0["f"], ht0["b"]):
        init_ht(t_)
    for s in ("0f", "0b"):
        nc.sync.dma_start(out=ht8[s], in_=dins["hti8"])
    nc.sync.dma_start(out=wh0_8["b"], in_=dins["wh0b8"])
    wh0 = {}
    s1T = {}
    for nm in ("s", "e"):
        st = sepool.tile([128, 2, L], BF16, name=nm + "1T", tag=nm + "1T")
        nc.sync.dma_start(out=st[F - 128:F - 127, 1, :],
                          in_=ones_c.rearrange("p b t -> p (b t)")[:, 0:L])
        s1T[nm] = st
    whw = {}

    def deferred_b_loads():
        # issued behind the wx0b reload ON THE SAME (gpsimd) DMA queue, so
        # none of it can jump ahead of wx0b on the shared DMA engines.  The
        # bf16 wh0 is only read by the final (bf16) iteration ~4 iterations
        # later; everything else here is needed from phase C onward.
        g = nc.gpsimd
        wh0["f"] = _load_w(nc, whpool, dins["wh0f"], NKH, 3 * GP, "wh0f", eng=g)
        wh0["b"] = _load_w(nc, whpool, dins["wh0b"], NKH, 3 * GP, "wh0b", eng=g)
        g.dma_start(out=wh1_8["f"], in_=dins["wh1f8"])
        g.dma_start(out=wh1_8["b"], in_=dins["wh1b8"])
        for t_ in (ht1["f"], ht1["b"]):
            g.dma_start(out=t_, in_=dins["hti"])
        for s in ("1f", "1b"):
            g.dma_start(out=ht8[s], in_=dins["hti8"])
        whw["f"] = _load_w(nc, endw, dins["whwf"], NKH, 2 * GP, "whwf", eng=g)
        whw["b"] = _load_w(nc, endw, dins["whwb"], NKH, 2 * GP, "whwb", eng=g)
        wh1["f"] = _load_w(nc, wh1pool, dins["wh1f"], NKH, 3 * GP, "wh1f", eng=g)
        wh1["b"] = _load_w(nc, wh1pool, dins["wh1b"], NKH, 3 * GP, "wh1b", eng=g)

    def psum_tile():
        return ppool.tile([128, 4, GP], F32, name="pz", tag="pz")

    xt0 = {}

    def phase_a_dir(s, wt):
        store = xtpool.tile([128, NM, GP], BF16, name="xt0" + s, tag="xt" + s)
        for grp in range(3):
            pz = psum_tile()
            # k outermost: each weight K-chunk is consumed as soon as its DMA
            # lands instead of demanding all chunks for the first output
            for k in range(NKD):
                for mi in range(4):
                    m = grp * 4 + mi
                    mov = xt_sb[:, k, :] if s == "f" else xt_rev[:, k, :, :]
                    nc.tensor.matmul(pz[:, mi, :], wt[:, k, m * 128:(m + 1) * 128],
                                     mov, start=(k == 0), stop=(k == NKD - 1))
            nc.scalar.copy(store[:, grp * 4:(grp + 1) * 4, :], pz)
        xt0[s] = store

    def jacobi_iter(stream, it):
        """Emit one Jacobi iteration for one direction stream.
        stream = (wh_dict, key, wh_pairs_fp8, xs, ht_bf16, ht_fp8, si);
        ht is [128, 4, 2, 257], pre-initialized (slot 0 = h0, ones rail at
        [16, 3], zeros).

        Iterations 0..K-2 run the recurrence matmuls in fp8 DoubleRow mode
        (2 K-tiles per instruction at 0.5 cycles/row) against the fp8 h-state;
        weights are pre-scaled by W8SCALE and the activation divides it back
        out.  The x~ injection stays bf16 (its accuracy persists into the
        fixed point).  The last iteration runs fully in bf16: the fp8
        quantization noise of earlier iterates contracts by ~4x per iteration,
        so only bf16-level noise survives in the final h."""
        wh_d, wh_k, wh_p8, xs, ht, h8, si = stream
        fp8 = it < K_ITERS - 1
        mov = h8 if fp8 else ht
        # the last fp8 iteration feeds the bf16 one: write bf16 h
        wout = ht if it >= K_ITERS - 2 else h8
        ss = str(si)
        I = trans.tile([128, 4, BL, T], BF16, name="I" + ss, tag="I" + ss)
        Gt = trans.tile([128, 4, BL, T], BF16, name="Gt" + ss, tag="Gt" + ss)
        O = trans.tile([128, 4, BL, T], BF16, name="O" + ss, tag="O" + ss)
        for g, (dst, fn) in enumerate(((I, AF.Sigmoid), (Gt, AF.Tanh),
                                       (O, AF.Sigmoid))):
            pz = psum_tile()
            for mi in range(4):
                m = g * 4 + mi
                nc.tensor.matmul(pz[:, mi, :], ident128 if fp8 else ident,
                                 xs[:, m, :], start=True, stop=False)
                if fp8:
                    for pair in range(2):
                        nc.tensor.matmul(
                            pz[:, mi, :],
                            wh_p8[:, pair, :, m * 128:(m + 1) * 128],
                            mov[:, 2 * pair:2 * pair + 2, :, 0:T],
                            start=False, stop=(pair == 1), perf_mode=DR)
                else:
                    for k in range(NKH):
                        nc.tensor.matmul(pz[:, mi, :],
                                         wh_d[wh_k][:, k, m * 128:(m + 1) * 128],
                                         mov[:, k, :, 0:T],
                                         start=False, stop=(k == NKH - 1))
            nc.scalar.activation(dst, pz.rearrange("p m (b t) -> p m b t", b=BL),
                                 fn, scale=(1.0 / W8SCALE) if fp8 else 1.0)
        nc.vector.tensor_mul(Gt, I, Gt)                     # b_t = i * g
        nc.vector.tensor_scalar(out=I, in0=I, scalar1=-1.0, scalar2=1.0,
                                op0=ALU.mult, op1=ALU.add)
        for k in range(4):                                   # c scan per (chunk, b)
            for b in range(BL):
                # in-place over Gt: the scan reads data1[t] before
                # writing out[t], so out may alias data1
                nc.vector.tensor_tensor_scan(
                    out=Gt[:, k, b, :], data0=I[:, k, b, :], data1=Gt[:, k, b, :],
                    initial=c0sb[:, k:k + 1],
                    op0=ALU.mult, op1=ALU.add)
        nc.scalar.activation(Gt, Gt, AF.Tanh)
        nc.vector.tensor_mul(wout[:, 0:3, :, 1:T + 1], Gt[:, 0:3], O[:, 0:3])
        nc.vector.tensor_mul(wout[0:16, 3, :, 1:T + 1], Gt[0:16, 3], O[0:16, 3])

    def lstm_jacobi_pair(streams, skip=()):
        for it in range(K_ITERS):
            for stream in streams:
                if (stream[-1], it) not in skip:
                    jacobi_iter(stream, it)

    # -------- phases A+B interleaved --------
    # A-f; then Jacobi-L0 f iteration 0 keeps the PE busy while the wx0b
    # reload (WAR on the shared wx0 buffer) and the deferred weight loads
    # drain on the gpsimd DMA queue; then A-b; then the remaining iterations.
    streamBf = (wh0, "f", wh0_8["f"], xt0, ht0["f"], ht8["0f"], 0)
    streamBb = (wh0, "b", wh0_8["b"], xt0, ht0["b"], ht8["0b"], 1)
    phase_a_dir("f", wx0f)
    streamBf = streamBf[:3] + (xt0["f"],) + streamBf[4:]
    jacobi_iter(streamBf, 0)
    wx0b = _load_w(nc, xpool, dins["wx0b"], NKD, 3 * GP, "wx0", nsplit=3,
                   eng=nc.gpsimd)
    deferred_b_loads()
    phase_a_dir("b", wx0b)
    xpool.release()
    streamBb = streamBb[:3] + (xt0["b"],) + streamBb[4:]
    lstm_jacobi_pair([streamBf, streamBb], skip={(0, 0)})
    whpool.release()

    # reversed-time copies (the ones rail at [16, 3] copies over too)
    nc.vector.tensor_copy(ht0["fr"][:, :, :, 1:T + 1], ht0["f"][:, :, :, T:0:-1])
    nc.vector.tensor_copy(ht0["br"][:, :, :, 1:T + 1], ht0["b"][:, :, :, T:0:-1])

    # -------- phase C: layer-1 x_tilde --------
    # Tile creation order steers which freed region each buffer reuses:
    # slot 0 -> the fp8 wh0 pair (dead after fp8 iteration 3), slots 1/2 ->
    # bf16 wh0 (dead at B's end), slot 3 -> the phase-A x/wx region (dead
    # since A).  So the f-direction pair (ff, fb) can stream in DURING
    # phase B, and bf/bb stream during the f-direction matmuls.
    wx1pool = tc.alloc_tile_pool(name="wx1", bufs=1)
    wx1t = {}
    for nm in ("ff", "bf", "bb", "fb"):
        wx1t[nm] = wx1pool.tile([128, NKH, 3 * GP], BF16, name="wx1" + nm,
                                tag="wx1" + nm)
    for nm in ("ff", "fb", "bf", "bb"):
        nc.gpsimd.dma_start(out=wx1t[nm], in_=dins["wx1" + nm])
    wx1 = {"f": (wx1t["ff"], wx1t["fb"]), "b": (wx1t["bf"], wx1t["bb"])}

    xt1 = {}
    for s, (hfmov, hbmov) in (("f", (ht0["f"], ht0["br"])), ("b", (ht0["fr"], ht0["b"]))):
        wtf, wtb = wx1[s]
        store = xtpool.tile([128, NM, GP], BF16, name="xt1" + s, tag="xt" + s)
        pairs = [(wtf, hfmov, k) for k in range(NKH)] + [(wtb, hbmov, k) for k in range(NKH)]
        for grp in range(3):
            pz = psum_tile()
            for mi in range(4):
                m = grp * 4 + mi
                for pi, (wt, mov, k) in enumerate(pairs):
                    nc.tensor.matmul(pz[:, mi, :], wt[:, k, m * 128:(m + 1) * 128],
                                     mov[:, k, :, 1:T + 1],
                                     start=(pi == 0), stop=(pi == 7))
            nc.scalar.copy(store[:, grp * 4:(grp + 1) * 4, :], pz)
        xt1[s] = store
    wx1pool.release()
    ht0tmp.release()

    # -------- phase D: layer-1 recurrences (both directions interleaved) -----
    lstm_jacobi_pair([(wh1, "f", wh1_8["f"], xt1["f"], ht1["f"], ht8["1f"], 0),
                      (wh1, "b", wh1_8["b"], xt1["b"], ht1["b"], ht8["1b"], 1)])
    wh1pool.release()
    nc.vector.tensor_copy(ht1["br"][:, :, :, 1:T + 1], ht1["b"][:, :, :, T:0:-1])
    xtpool.release()

    # -------- phase E: highway gate + blend (in place over ht0 f/br slots) ----
    outT = {}
    pairs = [(whw["f"], ht1["f"], k) for k in range(NKH)] + \
            [(whw["b"], ht1["br"], k) for k in range(NKH)]
    for half, (h1, h0) in (("f", (ht1["f"], ht0["f"])), ("b", (ht1["br"], ht0["br"]))):
        pz = psum_tile()
        for mi in range(4):
            m = (0 if half == "f" else 4) + mi
            for pi, (wt, mov, k) in enumerate(pairs):
                nc.tensor.matmul(pz[:, mi, :], wt[:, k, m * 128:(m + 1) * 128],
                                 mov[:, k, :, 1:T + 1],
                                 start=(pi == 0), stop=(pi == 7))
        gate = trans.tile([128, 4, BL, T], BF16, name="gate", tag="I0")
        nc.scalar.activation(gate, pz.rearrange("p m (b t) -> p m b t", b=BL), AF.Sigmoid)
        tmp = trans.tile([128, 4, BL, T], BF16, name="tmpb", tag="Gt0")
        hsl = h0[:, :, :, 1:T + 1]
        nc.vector.tensor_sub(tmp, h1[:, :, :, 1:T + 1], hsl)
        nc.vector.tensor_mul(tmp, gate, tmp)
        # the final write skips partition 16 of chunk 3 so the ones rail from
        # the init image survives for the projection bias rows
        nc.vector.tensor_add(hsl[:, 0:3], hsl[:, 0:3], tmp[:, 0:3])
        nc.vector.tensor_add(hsl[0:16, 3], hsl[0:16, 3], tmp[0:16, 3])
        outT[half] = h0
    ht1pool.release()
    trans.release()

    # -------- phase F: s/e projections --------
    latew = tc.alloc_tile_pool(name="latew", bufs=1)
    wse = {}
    for nm in ("s", "e"):
        wse[nm] = {"f": _load_w(nc, latew, dins["w" + nm + "f"], NKH, F, "w" + nm + "f"),
                   "b": _load_w(nc, latew, dins["w" + nm + "b"], NKH, F, "w" + nm + "b")}
    ut = _load_w(nc, latew, dins["upk"], 2, C * 256, "upk")
    for nm in ("s", "e"):
        wf, wb = wse[nm]["f"], wse[nm]["b"]
        st = s1T[nm]
        prs = [(wf, outT["f"], k) for k in range(NKH)] + [(wb, outT["b"], k) for k in range(NKH)]
        pz = psum_tile()
        for mi, (ma, mb) in enumerate(((0, 128), (128, F))):
            for pi, (wt, mov, k) in enumerate(prs):
                nc.tensor.matmul(pz[0:mb - ma, mi, :], wt[:, k, ma:mb],
                                 mov[:, k, :, 1:T + 1],
                                 start=(pi == 0), stop=(pi == 7))
        nc.scalar.copy(st[:, 0, :], pz[:, 0, :])
        nc.scalar.copy(st[0:F - 128, 1, :], pz[0:F - 128, 1, :])

    # -------- phase G: biaffine part 1: tmp[(c,j), (b,t)] --------
    biapool = tc.alloc_tile_pool(name="bia", bufs=1)
    smov = [s1T["s"][:, 0, :], s1T["s"][0:F + 1 - 128, 1, :]]
    ut_t = [ut[:, 0, :], ut[0:F + 1 - 128, 1, :]]
    tmpT = biapool.tile([128, 16, GP], BF16, name="tmpT", tag="tmpT")
    for grp in range(4):
        pz = psum_tile()
        for mi in range(4):
            m = grp * 4 + mi
            for k in range(2):
                nc.tensor.matmul(pz[:, mi, :], ut_t[k][:, m * 128:(m + 1) * 128],
                                 smov[k], start=(k == 0), stop=(k == 1))
        nc.scalar.copy(tmpT[:, grp * 4:(grp + 1) * 4, :], pz)

    # -------- phase H: biaffine part 2 + output assembly --------
    emov0 = s1T["e"][:, 0, :].rearrange("p (b t) -> p b t", b=BL)
    emov1 = s1T["e"][0:F + 1 - 128, 1, :].rearrange("p (b t) -> p b t", b=BL)
    ssbpool = tc.alloc_tile_pool(name="osb", bufs=2)
    for bi in range(BL):
        for xt_i in range(2):
            osb = ssbpool.tile([128, T, C], BF16, name="osb", tag="osb")
            pz = psum_tile()
            for c in range(C):
                xsl = slice(bi * T + xt_i * 128, bi * T + xt_i * 128 + 128)
                po = pz[:, c // 2, (c % 2) * T:(c % 2) * T + T]
                nc.tensor.matmul(po, tmpT[:, 2 * c, xsl], emov0[:, bi, :],
                                 start=True, stop=False)
                nc.tensor.matmul(po, tmpT[0:F + 1 - 128, 2 * c + 1, xsl],
                                 emov1[:, bi, :], start=False, stop=True)
            # one merged copy per block: psum [128, 4, 2, 256] -> osb [t, c]
            eng = nc.vector if (bi * 2 + xt_i) % 2 == 0 else nc.scalar
            if eng is nc.vector:
                nc.vector.tensor_copy(
                    osb.rearrange("p t (chi clo) -> p chi clo t", clo=2),
                    pz.rearrange("p m (clo t) -> p m clo t", clo=2))
            else:
                nc.scalar.copy(
                    osb.rearrange("p t (chi clo) -> p chi clo t", clo=2),
                    pz.rearrange("p m (clo t) -> p m clo t", clo=2))
            nc.sync.dma_start(out=out_d[bi, xt_i * 128:(xt_i + 1) * 128, :, :], in_=osb)
    ssbpool.release()
    biapool.release()
    latew.release()
    ht0pool.release()
    sepool.release()
    endw.release()
    ppool.release()
    const.release()


# ------------------------------------------------------------------ entry point

TRACE = False          # set True (from test harnesses) to capture an NTFF profile
LAST_RESULT = None     # BassKernelResults of the most recent run


def kernel(**inputs) -> np.ndarray:
    global LAST_RESULT
    if "nc" not in _CACHE:
        _CACHE["nc"] = _build_program()
    nc = _CACHE["nc"]
    in_maps = _pack_inputs(inputs)
    try:
        res = run_bass_kernel_spmd(nc, in_maps, core_ids=list(range(NCORES)),
                                   trace=TRACE)
    except ModuleNotFoundError:
        # no NTFF profile hook in this container; run without tracing
        res = run_bass_kernel_spmd(nc, in_maps, core_ids=list(range(NCORES)))
    LAST_RESULT = res
    out = np.concatenate([np.asarray(res.results[c]["out"]) for c in range(NCORES)],
                         axis=0)
    return np.ascontiguousarray(out.astype(np.float32))


if __name__ == "__main__":
    raise SystemExit("use test.py")
